# revision 11
# baseline (speedup 1.0000x reference)
"""CFConv (SchNet continuous-filter convolution) Trainium2 kernel.

Math (per molecule b):
    rbf[b,i,j,r] = exp(-gamma * (dist[b,i,j] - r*res)^2),  r = 0..299
    f = softplus(rbf @ W1 + b1); f = softplus(f @ W2 + b2)
    out[b,j,c] = sum_i h[b,i,c] * f[b,i,j,c]

Device-level reformulations:
  * dist < 10.0 and gamma=10 => centers r >= 128 (c_r >= 12.8) give
    exp(<= -78) ~ 1e-34: identically negligible in fp32. K: 300 -> 128.
  * -g(d-c)^2 = (-g)*d^2 + (2gc)*d + (-g c^2): the first two terms form a
    K=2 matmul over host-provided rows [d^2; d]; the per-r constant is the
    per-partition bias of the Exp activation.
  * softplus(x) = ln(exp(x) + 1) via Exp then Ln activations (both live in
    the same ACT table set; no native softplus table is deployed).
  * Elements are flattened in (b, j, i) order so the final contraction
    over i is a native inner-axis vector reduce per 512-element chunk.
  * Channel dim is 64; two 512-element chunks are stacked to fill all 128
    partitions for mm1/softplus/mm2/softplus/mul/reduce.

Raw Bass (no Tile): the deployed walrus accepts at most one sync-wait per
instruction, so all cross-engine deps are standalone single-condition
wait_ge instructions; buffers are double-buffered with parity t % 2.

Sharding: data-parallel over mb across 8 cores (4 molecules/core), params
replicated. No collectives; host splits inputs and reassembles outputs.
"""

import numpy as np

MB, ATOM, HD = 32, 64, 64
R = 128                     # effective RBF count (of 300)
GAMMA, RES = 10.0, 0.1
NCORES = 8
MBC = MB // NCORES          # molecules per core
E = MBC * ATOM * ATOM       # flattened (b, j, i) elements per core
CH = 512                    # e-chunk (one PSUM bank col-width)
NCHUNK = E // CH
NPAIR = NCHUNK // 2

_CACHE = {}


def build_bass():
    from contextlib import ExitStack

    import concourse.bass as bass
    from concourse import mybir

    f32 = mybir.dt.float32
    AF = mybir.ActivationFunctionType

    nc = bass.Bass()
    dd = nc.declare_dram_parameter("dd", [2, E], f32, isOutput=False)
    coef = nc.declare_dram_parameter("coef", [2, R], f32, isOutput=False)
    bexp = nc.declare_dram_parameter("bexp", [R, 1], f32, isOutput=False)
    w1 = nc.declare_dram_parameter("w1", [R, HD], f32, isOutput=False)
    b1p = nc.declare_dram_parameter("b1p", [128, 1], f32, isOutput=False)
    w2 = nc.declare_dram_parameter("w2", [2 * HD, HD], f32, isOutput=False)
    b2p = nc.declare_dram_parameter("b2p", [128, 1], f32, isOutput=False)
    hs = nc.declare_dram_parameter("hs", [128, MBC * ATOM], f32, isOutput=False)
    res = nc.declare_dram_parameter("res", [128, NPAIR * 8], f32, isOutput=True)

    with ExitStack() as ctx:
        en = ctx.enter_context

        dd_sb = en(nc.sbuf_tensor("dd_sb", [2, E], f32))
        coef_sb = en(nc.sbuf_tensor("coef_sb", [2, R], f32))
        bexp_sb = en(nc.sbuf_tensor("bexp_sb", [R, 1], f32))
        w1_sb = en(nc.sbuf_tensor("w1_sb", [R, HD], f32))
        b1p_sb = en(nc.sbuf_tensor("b1p_sb", [128, 1], f32))
        w2_sb = en(nc.sbuf_tensor("w2_sb", [2 * HD, HD], f32))
        b2p_sb = en(nc.sbuf_tensor("b2p_sb", [128, 1], f32))
        hs_sb = en(nc.sbuf_tensor("hs_sb", [128, MBC * ATOM], f32))
        res_sb = en(nc.sbuf_tensor("res_sb", [128, NPAIR * 8], f32))

        rbf_sb = [en(nc.sbuf_tensor(f"rbf{i}", [128, 2 * CH], f32)) for i in (0, 1)]
        u1_sb = [en(nc.sbuf_tensor(f"u1_{i}", [128, CH], f32)) for i in (0, 1)]
        f1_sb = [en(nc.sbuf_tensor(f"f1_{i}", [128, CH], f32)) for i in (0, 1)]
        u2_sb = [en(nc.sbuf_tensor(f"u2_{i}", [128, CH], f32)) for i in (0, 1)]
        f2_sb = [en(nc.sbuf_tensor(f"f2_{i}", [128, CH], f32)) for i in (0, 1)]
        prod_sb = [en(nc.sbuf_tensor(f"prod{i}", [128, 8, ATOM], f32)) for i in (0, 1)]

        exp_ps = [en(nc.psum_tensor(f"expps{i}", [128, 2 * CH], f32)) for i in (0, 1)]
        mm1_ps = [en(nc.psum_tensor(f"mm1ps{i}", [128, CH], f32)) for i in (0, 1)]
        mm2_ps = [en(nc.psum_tensor(f"mm2ps{i}", [128, CH], f32)) for i in (0, 1)]

        dma_sem = en(nc.semaphore("dma_sem"))
        pe_sem = en(nc.semaphore("pe_sem"))
        act_sem = en(nc.semaphore("act_sem"))
        dve_sem = en(nc.semaphore("dve_sem"))

        LOADS = 8  # input DMA transfers

        # ---- software-pipelined schedule ----
        # PE order:  exp(0) | per t: exp(t+1), mm1(t), mm2(t)
        # ACT order: rbf(0) | per t: u1(t), f2(t-1), f1(t), rbf(t+1), u2(t)
        #            | f2(15)
        # (same-engine dependent ACT ops are separated by an independent op;
        #  explicit self-waits satisfy the deep-pipeline RAW hazard cheaply)
        def seq_counts(names):
            return {n: i + 1 for i, n in enumerate(names)}

        pe_ops = ["exp0a", "exp0b"]
        for t in range(NPAIR):
            if t + 1 < NPAIR:
                pe_ops += [f"exp{t + 1}a", f"exp{t + 1}b"]
            pe_ops += [f"mm1_{t}a", f"mm1_{t}b", f"mm2_{t}a", f"mm2_{t}b"]
        PEC = seq_counts(pe_ops)

        act_ops = ["rbf0"]
        for t in range(NPAIR):
            act_ops.append(f"u1_{t}")
            if t >= 1:
                act_ops.append(f"f2_{t - 1}")
            act_ops.append(f"f1_{t}")
            if t + 1 < NPAIR:
                act_ops.append(f"rbf{t + 1}")
            act_ops.append(f"u2_{t}")
        act_ops.append(f"f2_{NPAIR - 1}")
        ACTC = seq_counts(act_ops)

        dve_ops = []
        for t in range(NPAIR):
            dve_ops += [f"mul{t}", f"red{t}"]
        DVEC = seq_counts(dve_ops)

        with nc.Block() as block:

            @block.gpsimd
            def _(g):
                for dst, src in [
                    (dd_sb, dd), (coef_sb, coef), (bexp_sb, bexp), (w1_sb, w1),
                    (b1p_sb, b1p), (w2_sb, w2), (b2p_sb, b2p), (hs_sb, hs),
                ]:
                    g.dma_start(dst[:], src[:]).then_inc(dma_sem, 16)
                # output store after the last reduce
                g.wait_ge(dve_sem, DVEC[f"red{NPAIR - 1}"])
                g.dma_start(res[:], res_sb[:]).then_inc(dma_sem, 16)
                g.wait_ge(dma_sem, 16 * (LOADS + 1))

            def emit_exp_mm(pe, k):
                p = k % 2
                for half, sfx in ((0, "a"), (1, "b")):
                    q = 2 * k + half
                    pe.matmul(
                        exp_ps[p][:, half * CH : (half + 1) * CH],
                        coef_sb[:],
                        dd_sb[:, q * CH : (q + 1) * CH],
                        start=True, stop=True,
                    ).then_inc(pe_sem, 1)

            @block.tensor
            def _(pe):
                pe.wait_ge(dma_sem, 16 * LOADS)
                emit_exp_mm(pe, 0)
                for t in range(NPAIR):
                    p = t % 2
                    if t + 1 < NPAIR:
                        if t - 1 >= 0:  # exp_ps[(t+1)%2] freed by rbf(t-1)
                            pe.wait_ge(act_sem, ACTC[f"rbf{t - 1}"])
                        emit_exp_mm(pe, t + 1)
                    pe.wait_ge(act_sem, ACTC[f"rbf{t}"])
                    pe.matmul(
                        mm1_ps[p][0:64, :], w1_sb[:], rbf_sb[p][:, 0:CH],
                        start=True, stop=True,
                    ).then_inc(pe_sem, 1)
                    pe.matmul(
                        mm1_ps[p][64:128, :], w1_sb[:], rbf_sb[p][:, CH : 2 * CH],
                        start=True, stop=True,
                    ).then_inc(pe_sem, 1)
                    pe.wait_ge(act_sem, ACTC[f"f1_{t}"])
                    pe.matmul(
                        mm2_ps[p][0:64, :], w2_sb[0:64, :], f1_sb[p][0:64, :],
                        start=True, stop=True,
                    ).then_inc(pe_sem, 1)
                    pe.matmul(
                        mm2_ps[p][64:128, :], w2_sb[64:128, :], f1_sb[p][64:128, :],
                        start=True, stop=True,
                    ).then_inc(pe_sem, 1)

            @block.scalar
            def _(act):
                act.wait_ge(dma_sem, 16 * LOADS)

                def rbf_act(k):
                    p = k % 2
                    act.wait_ge(pe_sem, PEC[f"exp{k}b"])
                    act.activation(
                        rbf_sb[p][:], exp_ps[p][:], AF.Exp, bias=bexp_sb[:]
                    ).then_inc(act_sem, 1)

                def f2_act(k):
                    p = k % 2
                    act.wait_ge(act_sem, ACTC[f"u2_{k}"])
                    if k >= 2:  # f2_sb[p] freed by DVE mul(k-2)
                        act.wait_ge(dve_sem, DVEC[f"mul{k - 2}"])
                    act.activation(
                        f2_sb[p][:], u2_sb[p][:], AF.Ln, bias=1.0
                    ).then_inc(act_sem, 1)

                rbf_act(0)
                for t in range(NPAIR):
                    p = t % 2
                    act.wait_ge(pe_sem, PEC[f"mm1_{t}b"])
                    act.activation(
                        u1_sb[p][:], mm1_ps[p][:], AF.Exp, bias=b1p_sb[:]
                    ).then_inc(act_sem, 1)
                    if t >= 1:
                        f2_act(t - 1)
                    act.wait_ge(act_sem, ACTC[f"u1_{t}"])
                    act.activation(
                        f1_sb[p][:], u1_sb[p][:], AF.Ln, bias=1.0
                    ).then_inc(act_sem, 1)
                    if t + 1 < NPAIR:
                        rbf_act(t + 1)
                    act.wait_ge(pe_sem, PEC[f"mm2_{t}b"])
                    act.activation(
                        u2_sb[p][:], mm2_ps[p][:], AF.Exp, bias=b2p_sb[:]
                    ).then_inc(act_sem, 1)
                f2_act(NPAIR - 1)

            @block.vector
            def _(ve):
                ve.wait_ge(dma_sem, 16 * LOADS)
                for t in range(NPAIR):
                    p = t % 2
                    b = t // (NPAIR // MBC)
                    ve.wait_ge(act_sem, ACTC[f"f2_{t}"])
                    if t >= 2:  # prod_sb[p] freed by red(t-2)
                        ve.wait_ge(dve_sem, DVEC[f"red{t - 2}"])
                    ve.tensor_mul(
                        prod_sb[p][:],
                        f2_sb[p][:].rearrange("p (j i) -> p j i", i=ATOM),
                        hs_sb[:, b * ATOM : (b + 1) * ATOM][:, None, :].broadcast_to(
                            [128, 8, ATOM]
                        ),
                    ).then_inc(dve_sem, 1)
                    ve.wait_ge(dve_sem, DVEC[f"mul{t}"])
                    ve.reduce_sum(
                        res_sb[:, t * 8 : (t + 1) * 8],
                        prod_sb[p][:],
                        axis=mybir.AxisListType.X,
                    ).then_inc(dve_sem, 1)

    return nc


def host_prep(h, dist, W1, b1, W2, b2):
    """Build per-core input maps (numpy only, layout/index prep)."""
    f4 = np.float32
    r = np.arange(R, dtype=f4)
    coef = np.stack([np.full(R, -GAMMA, f4), (2.0 * GAMMA * RES * r).astype(f4)])
    bexp = (-GAMMA * (RES * r) ** 2).astype(f4)[:, None]
    w1 = np.ascontiguousarray(W1[:R].astype(f4))
    b1p = np.concatenate([b1, b1]).astype(f4)[:, None]
    w2 = np.ascontiguousarray(np.concatenate([W2, W2], 0).astype(f4))
    b2p = np.concatenate([b2, b2]).astype(f4)[:, None]

    in_maps = []
    for g in range(NCORES):
        dist_c = dist[g * MBC : (g + 1) * MBC].astype(f4)
        dperm = np.ascontiguousarray(dist_c.transpose(0, 2, 1)).reshape(-1)  # (b,j,i)
        ddv = np.ascontiguousarray(np.stack([dperm * dperm, dperm]))
        h_c = h[g * MBC : (g + 1) * MBC].astype(f4)
        ht = np.ascontiguousarray(h_c.transpose(2, 0, 1)).reshape(HD, MBC * ATOM)
        hsv = np.ascontiguousarray(np.concatenate([ht, ht], 0))
        in_maps.append(
            {
                "dd": ddv, "coef": coef, "bexp": bexp, "w1": w1,
                "b1p": b1p, "w2": w2, "b2p": b2p, "hs": hsv,
            }
        )
    return in_maps


def decode_res(res_np):
    """res [128, 128] -> out_core [MBC, ATOM(j), HD(c)].

    res[cc, t*8+jl]: b = t//4, sig = t%4, j = 16*sig + 8*(cc>=64) + jl,
    c = cc % 64.
    """
    r5 = res_np.reshape(2, HD, MBC, NPAIR // MBC, 8)  # [half, c, b, sig, jl]
    return np.ascontiguousarray(r5.transpose(2, 3, 0, 4, 1)).reshape(MBC, ATOM, HD)


def kernel(h, dist, W1, b1, W2, b2):
    from concourse.bass_utils import run_bass_kernel_spmd

    if "nc" not in _CACHE:
        _CACHE["nc"] = build_bass()
    nc = _CACHE["nc"]
    in_maps = host_prep(h, dist, W1, b1, W2, b2)
    out = run_bass_kernel_spmd(nc, in_maps, list(range(NCORES)))
    cores = [decode_res(out.results[g]["res"]) for g in range(NCORES)]
    return np.concatenate(cores, axis=0).astype(np.float32)


# revision 16
# speedup vs baseline: 1.6191x; 1.6191x over previous
"""CFConv (SchNet continuous-filter convolution) Trainium2 kernel.

Math (per molecule b):
    rbf[b,i,j,r] = exp(-gamma * (dist[b,i,j] - r*res)^2),  r = 0..299
    f = softplus(rbf @ W1 + b1); f = softplus(f @ W2 + b2)
    out[b,j,c] = sum_i h[b,i,c] * f[b,i,j,c]

Device-level reformulations:
  * dist < 10.0 and gamma=10 => centers r >= 128 (c_r >= 12.8) give
    exp(<= -78) ~ 1e-34: identically negligible in fp32. K: 300 -> 128.
  * -g(d-c)^2 = (-g)*d^2 + (2gc)*d + (-g c^2): the first two terms form a
    matmul over host-provided rows; the per-r constant is the per-partition
    bias of the Exp activation. fp32 matmul on this PE runs in slow
    LOW_HIGH emulation (~2.1us per 512-col op), so all matmuls use bf16:
      - expansion: d and d^2 are each split into 3 bf16 components (K=6).
        The coefficients -10 and 2r (integers < 256) are EXACT in bf16,
        so every product is exact; residual ~2e-4 in the exponent.
      - mm1: W1 split hi+lo bf16 (K=2x128, PSUM-accumulated); rbf in bf16.
      - mm2: W2 split hi+lo bf16; f1 stored bf16 after a range shift:
        f1' = softplus(x1) - kappa via Ln(e^-k * u1 + e^-k), which halves
        the bf16 absolute error; kappa is compensated in b2.
  * softplus(x) = ln(exp(x) + 1) via Exp then Ln activations (both live in
    the same ACT table set; no native softplus table is deployed).
  * Elements are flattened in (b, j, i) order so the final contraction
    over i is a native inner-axis vector reduce per 512-element chunk.
  * Channel dim is 64; two 512-element chunks are stacked to fill all 128
    partitions for mm1/softplus/mm2/softplus/mul/reduce.

Raw Bass (no Tile): the deployed walrus accepts at most one sync-wait per
instruction, so all cross-engine deps are standalone single-condition
wait_ge instructions; buffers are double-buffered with parity t % 2, and
same-engine dependent ACT ops are separated by an independent op so the
self-wait is nearly free.

Sharding: data-parallel over mb across 8 cores (4 molecules/core), params
replicated. No collectives; host splits inputs and reassembles outputs.
"""

import numpy as np

MB, ATOM, HD = 32, 64, 64
R = 128                     # effective RBF count (of 300)
GAMMA, RES = 10.0, 0.1
KAPPA = 0.875               # f1 range shift (exact in bf16)
NCORES = 8
MBC = MB // NCORES          # molecules per core
E = MBC * ATOM * ATOM       # flattened (b, j, i) elements per core
CH = 512                    # e-chunk (one PSUM bank col-width)
NCHUNK = E // CH
NPAIR = NCHUNK // 2

_CACHE = {}


def build_bass():
    from contextlib import ExitStack

    import concourse.bass as bass
    from concourse import mybir

    f32 = mybir.dt.float32
    bf16 = mybir.dt.bfloat16
    AF = mybir.ActivationFunctionType

    nc = bass.Bass()
    dd = nc.declare_dram_parameter("dd", [6, E], bf16, isOutput=False)
    coef = nc.declare_dram_parameter("coef", [6, R], bf16, isOutput=False)
    bexp = nc.declare_dram_parameter("bexp", [R, 1], f32, isOutput=False)
    w1h = nc.declare_dram_parameter("w1h", [R, HD], bf16, isOutput=False)
    w1l = nc.declare_dram_parameter("w1l", [R, HD], bf16, isOutput=False)
    b1p = nc.declare_dram_parameter("b1p", [128, 1], f32, isOutput=False)
    w2h = nc.declare_dram_parameter("w2h", [2 * HD, HD], bf16, isOutput=False)
    w2l = nc.declare_dram_parameter("w2l", [2 * HD, HD], bf16, isOutput=False)
    b2p = nc.declare_dram_parameter("b2p", [128, 1], f32, isOutput=False)
    hs = nc.declare_dram_parameter("hs", [128, MBC * ATOM], f32, isOutput=False)
    emk = nc.declare_dram_parameter("emk", [128, 1], f32, isOutput=False)
    res = nc.declare_dram_parameter("res", [128, NPAIR * 8], f32, isOutput=True)

    EMK = np.exp(-KAPPA).astype(np.float32) if False else float(np.exp(-KAPPA))

    with ExitStack() as ctx:
        en = ctx.enter_context

        dd_sb = en(nc.sbuf_tensor("dd_sb", [6, E], bf16))
        coef_sb = en(nc.sbuf_tensor("coef_sb", [6, R], bf16))
        bexp_sb = en(nc.sbuf_tensor("bexp_sb", [R, 1], f32))
        w1h_sb = en(nc.sbuf_tensor("w1h_sb", [R, HD], bf16))
        w1l_sb = en(nc.sbuf_tensor("w1l_sb", [R, HD], bf16))
        b1p_sb = en(nc.sbuf_tensor("b1p_sb", [128, 1], f32))
        w2h_sb = en(nc.sbuf_tensor("w2h_sb", [2 * HD, HD], bf16))
        w2l_sb = en(nc.sbuf_tensor("w2l_sb", [2 * HD, HD], bf16))
        b2p_sb = en(nc.sbuf_tensor("b2p_sb", [128, 1], f32))
        hs_sb = en(nc.sbuf_tensor("hs_sb", [128, MBC * ATOM], f32))
        emk_sb = en(nc.sbuf_tensor("emk_sb", [128, 1], f32))
        res_sb = en(nc.sbuf_tensor("res_sb", [128, NPAIR * 8], f32))

        rbf_sb = [en(nc.sbuf_tensor(f"rbf{i}", [128, 2 * CH], bf16)) for i in (0, 1)]
        u1_sb = [en(nc.sbuf_tensor(f"u1_{i}", [128, CH], f32)) for i in (0, 1)]
        f1_sb = [en(nc.sbuf_tensor(f"f1_{i}", [128, CH], bf16)) for i in (0, 1)]
        u2_sb = [en(nc.sbuf_tensor(f"u2_{i}", [128, CH], f32)) for i in (0, 1)]
        f2_sb = [en(nc.sbuf_tensor(f"f2_{i}", [128, CH], f32)) for i in (0, 1)]
        prod_sb = [en(nc.sbuf_tensor(f"prod{i}", [128, 8, ATOM], f32)) for i in (0, 1)]

        exp_ps = [en(nc.psum_tensor(f"expps{i}", [128, 2 * CH], f32)) for i in (0, 1)]
        mm1_ps = [en(nc.psum_tensor(f"mm1ps{i}", [128, CH], f32)) for i in (0, 1)]
        mm2_ps = [en(nc.psum_tensor(f"mm2ps{i}", [128, CH], f32)) for i in (0, 1)]

        dma_sem = en(nc.semaphore("dma_sem"))
        pe_sem = en(nc.semaphore("pe_sem"))
        act_sem = en(nc.semaphore("act_sem"))
        dve_sem = en(nc.semaphore("dve_sem"))

        LOADS = 11  # input DMA transfers

        # ---- software-pipelined schedule ----
        # PE order:  exp(0) | per t: exp(t+1), mm1(t), mm2(t)
        # ACT order: rbf(0) | per t: u1(t), f2(t-1), f1(t), rbf(t+1), u2(t)
        #            | f2(15)
        def seq_counts(names):
            return {n: i + 1 for i, n in enumerate(names)}

        pe_ops = ["exp0a", "exp0b"]
        for t in range(NPAIR):
            if t + 1 < NPAIR:
                pe_ops += [f"exp{t + 1}a", f"exp{t + 1}b"]
            pe_ops += [
                f"mm1h_{t}a", f"mm1l_{t}a", f"mm1h_{t}b", f"mm1l_{t}b",
                f"mm2h_{t}a", f"mm2l_{t}a", f"mm2h_{t}b", f"mm2l_{t}b",
            ]
        PEC = seq_counts(pe_ops)

        act_ops = ["rbf0"]
        for t in range(NPAIR):
            act_ops.append(f"u1_{t}")
            if t >= 1:
                act_ops.append(f"f2_{t - 1}")
            act_ops.append(f"f1_{t}")
            if t + 1 < NPAIR:
                act_ops.append(f"rbf{t + 1}")
            act_ops.append(f"u2_{t}")
        act_ops.append(f"f2_{NPAIR - 1}")
        ACTC = seq_counts(act_ops)

        dve_ops = []
        for t in range(NPAIR):
            dve_ops += [f"mul{t}", f"red{t}"]
        DVEC = seq_counts(dve_ops)

        with nc.Block() as block:

            @block.gpsimd
            def _(g):
                for dst, src in [
                    (dd_sb, dd), (coef_sb, coef), (bexp_sb, bexp),
                    (w1h_sb, w1h), (w1l_sb, w1l), (b1p_sb, b1p),
                    (w2h_sb, w2h), (w2l_sb, w2l), (b2p_sb, b2p),
                    (hs_sb, hs), (emk_sb, emk),
                ]:
                    g.dma_start(dst[:], src[:]).then_inc(dma_sem, 16)
                # output store after the last reduce
                g.wait_ge(dve_sem, DVEC[f"red{NPAIR - 1}"])
                g.dma_start(res[:], res_sb[:]).then_inc(dma_sem, 16)
                g.wait_ge(dma_sem, 16 * (LOADS + 1))

            def emit_exp_mm(pe, k):
                p = k % 2
                for half, sfx in ((0, "a"), (1, "b")):
                    q = 2 * k + half
                    pe.matmul(
                        exp_ps[p][:, half * CH : (half + 1) * CH],
                        coef_sb[:],
                        dd_sb[:, q * CH : (q + 1) * CH],
                        start=True, stop=True,
                    ).then_inc(pe_sem, 1)

            @block.tensor
            def _(pe):
                pe.wait_ge(dma_sem, 16 * LOADS)
                emit_exp_mm(pe, 0)
                for t in range(NPAIR):
                    p = t % 2
                    if t + 1 < NPAIR:
                        if t - 1 >= 0:  # exp_ps[(t+1)%2] freed by rbf(t-1)
                            pe.wait_ge(act_sem, ACTC[f"rbf{t - 1}"])
                        emit_exp_mm(pe, t + 1)
                    pe.wait_ge(act_sem, ACTC[f"rbf{t}"])
                    # mm1: K = 2x128 (W1 hi+lo), accumulate in PSUM.
                    # Each half's accumulation group closes before the next
                    # opens (one pending group per PSUM zero-region).
                    pe.matmul(
                        mm1_ps[p][0:64, :], w1h_sb[:], rbf_sb[p][:, 0:CH],
                        start=True, stop=False,
                    ).then_inc(pe_sem, 1)
                    pe.matmul(
                        mm1_ps[p][0:64, :], w1l_sb[:], rbf_sb[p][:, 0:CH],
                        start=False, stop=True,
                    ).then_inc(pe_sem, 1)
                    pe.matmul(
                        mm1_ps[p][64:128, :], w1h_sb[:], rbf_sb[p][:, CH : 2 * CH],
                        start=True, stop=False,
                    ).then_inc(pe_sem, 1)
                    pe.matmul(
                        mm1_ps[p][64:128, :], w1l_sb[:], rbf_sb[p][:, CH : 2 * CH],
                        start=False, stop=True,
                    ).then_inc(pe_sem, 1)
                    pe.wait_ge(act_sem, ACTC[f"f1_{t}"])
                    pe.matmul(
                        mm2_ps[p][0:64, :], w2h_sb[0:64, :], f1_sb[p][0:64, :],
                        start=True, stop=False,
                    ).then_inc(pe_sem, 1)
                    pe.matmul(
                        mm2_ps[p][0:64, :], w2l_sb[0:64, :], f1_sb[p][0:64, :],
                        start=False, stop=True,
                    ).then_inc(pe_sem, 1)
                    pe.matmul(
                        mm2_ps[p][64:128, :], w2h_sb[64:128, :], f1_sb[p][64:128, :],
                        start=True, stop=False,
                    ).then_inc(pe_sem, 1)
                    pe.matmul(
                        mm2_ps[p][64:128, :], w2l_sb[64:128, :], f1_sb[p][64:128, :],
                        start=False, stop=True,
                    ).then_inc(pe_sem, 1)

            @block.scalar
            def _(act):
                act.wait_ge(dma_sem, 16 * LOADS)

                def rbf_act(k):
                    p = k % 2
                    act.wait_ge(pe_sem, PEC[f"exp{k}b"])
                    act.activation(
                        rbf_sb[p][:], exp_ps[p][:], AF.Exp, bias=bexp_sb[:]
                    ).then_inc(act_sem, 1)

                def f2_act(k):
                    p = k % 2
                    act.wait_ge(act_sem, ACTC[f"u2_{k}"])
                    if k >= 2:  # f2_sb[p] freed by DVE mul(k-2)
                        act.wait_ge(dve_sem, DVEC[f"mul{k - 2}"])
                    act.activation(
                        f2_sb[p][:], u2_sb[p][:], AF.Ln, bias=1.0
                    ).then_inc(act_sem, 1)

                rbf_act(0)
                for t in range(NPAIR):
                    p = t % 2
                    act.wait_ge(pe_sem, PEC[f"mm1l_{t}b"])
                    act.activation(
                        u1_sb[p][:], mm1_ps[p][:], AF.Exp, bias=b1p_sb[:]
                    ).then_inc(act_sem, 1)
                    if t >= 1:
                        f2_act(t - 1)
                    act.wait_ge(act_sem, ACTC[f"u1_{t}"])
                    # f1' = ln(e^-k u1 + e^-k) = softplus(x1) - kappa, bf16
                    act.activation(
                        f1_sb[p][:], u1_sb[p][:], AF.Ln, bias=emk_sb[:], scale=EMK
                    ).then_inc(act_sem, 1)
                    if t + 1 < NPAIR:
                        rbf_act(t + 1)
                    act.wait_ge(pe_sem, PEC[f"mm2l_{t}b"])
                    act.activation(
                        u2_sb[p][:], mm2_ps[p][:], AF.Exp, bias=b2p_sb[:]
                    ).then_inc(act_sem, 1)
                f2_act(NPAIR - 1)

            @block.vector
            def _(ve):
                ve.wait_ge(dma_sem, 16 * LOADS)
                for t in range(NPAIR):
                    p = t % 2
                    b = t // (NPAIR // MBC)
                    ve.wait_ge(act_sem, ACTC[f"f2_{t}"])
                    if t >= 2:  # prod_sb[p] freed by red(t-2)
                        ve.wait_ge(dve_sem, DVEC[f"red{t - 2}"])
                    ve.tensor_mul(
                        prod_sb[p][:],
                        f2_sb[p][:].rearrange("p (j i) -> p j i", i=ATOM),
                        hs_sb[:, b * ATOM : (b + 1) * ATOM][:, None, :].broadcast_to(
                            [128, 8, ATOM]
                        ),
                    ).then_inc(dve_sem, 1)
                    ve.wait_ge(dve_sem, DVEC[f"mul{t}"])
                    ve.reduce_sum(
                        res_sb[:, t * 8 : (t + 1) * 8],
                        prod_sb[p][:],
                        axis=mybir.AxisListType.X,
                    ).then_inc(dve_sem, 1)

    return nc


def _split_bf16(x, n):
    """Split fp32 array into n bf16 components summing to ~x."""
    import ml_dtypes

    bf = ml_dtypes.bfloat16
    x = x.astype(np.float32)
    parts = []
    for _ in range(n):
        p = x.astype(bf)
        parts.append(p)
        x = x - p.astype(np.float32)
    return parts


def host_prep(h, dist, W1, b1, W2, b2):
    """Build per-core input maps (numpy only, layout/index prep)."""
    import ml_dtypes

    bf = ml_dtypes.bfloat16
    f4 = np.float32
    r = np.arange(R, dtype=f4)
    coef = np.stack(
        [np.full(R, -GAMMA, f4)] * 3 + [(2.0 * r).astype(f4)] * 3
    ).astype(bf)
    bexp = (-GAMMA * (RES * r) ** 2).astype(f4)[:, None]
    w1h_, w1l_ = _split_bf16(W1[:R], 2)
    b1p = np.concatenate([b1, b1]).astype(f4)[:, None]
    w2h_, w2l_ = _split_bf16(W2, 2)
    w2h2 = np.ascontiguousarray(np.concatenate([w2h_, w2h_], 0))
    w2l2 = np.ascontiguousarray(np.concatenate([w2l_, w2l_], 0))
    # kappa compensation: out2 = W2dev.T @ (f1 - kappa) + b2 + kappa*colsum(W2dev)
    w2dev = w2h_.astype(f4) + w2l_.astype(f4)
    b2c = (b2 + KAPPA * w2dev.sum(0)).astype(f4)
    b2p = np.concatenate([b2c, b2c]).astype(f4)[:, None]

    in_maps = []
    for g in range(NCORES):
        dist_c = dist[g * MBC : (g + 1) * MBC].astype(f4)
        dperm = np.ascontiguousarray(dist_c.transpose(0, 2, 1)).reshape(-1)  # (b,j,i)
        d2 = (dperm * dperm).astype(f4)
        ddv = np.ascontiguousarray(np.stack(_split_bf16(d2, 3) + _split_bf16(dperm, 3)))
        h_c = h[g * MBC : (g + 1) * MBC].astype(f4)
        ht = np.ascontiguousarray(h_c.transpose(2, 0, 1)).reshape(HD, MBC * ATOM)
        hsv = np.ascontiguousarray(np.concatenate([ht, ht], 0))
        in_maps.append(
            {
                "dd": ddv, "coef": coef, "bexp": bexp,
                "w1h": w1h_, "w1l": w1l_, "b1p": b1p,
                "w2h": w2h2, "w2l": w2l2, "b2p": b2p, "hs": hsv,
                "emk": np.full((128, 1), np.exp(-KAPPA), f4),
            }
        )
    return in_maps


def decode_res(res_np):
    """res [128, 128] -> out_core [MBC, ATOM(j), HD(c)].

    res[cc, t*8+jl]: b = t//4, sig = t%4, j = 16*sig + 8*(cc>=64) + jl,
    c = cc % 64.
    """
    r5 = res_np.reshape(2, HD, MBC, NPAIR // MBC, 8)  # [half, c, b, sig, jl]
    return np.ascontiguousarray(r5.transpose(2, 3, 0, 4, 1)).reshape(MBC, ATOM, HD)


def kernel(h, dist, W1, b1, W2, b2):
    from concourse.bass_utils import run_bass_kernel_spmd

    if "nc" not in _CACHE:
        _CACHE["nc"] = build_bass()
    nc = _CACHE["nc"]
    in_maps = host_prep(h, dist, W1, b1, W2, b2)
    out = run_bass_kernel_spmd(nc, in_maps, list(range(NCORES)))
    cores = [decode_res(out.results[g]["res"]) for g in range(NCORES)]
    return np.concatenate(cores, axis=0).astype(np.float32)


# revision 17
# speedup vs baseline: 1.8207x; 1.1245x over previous
"""CFConv (SchNet continuous-filter convolution) Trainium2 kernel.

Math (per molecule b):
    rbf[b,i,j,r] = exp(-gamma * (dist[b,i,j] - r*res)^2),  r = 0..299
    f = softplus(rbf @ W1 + b1); f = softplus(f @ W2 + b2)
    out[b,j,c] = sum_i h[b,i,c] * f[b,i,j,c]

Device-level reformulations:
  * dist < 10.0 and gamma=10 => centers r >= 128 (c_r >= 12.8) give
    exp(<= -78) ~ 1e-34: identically negligible in fp32. K: 300 -> 128.
  * -g(d-c)^2 = (-g)*d^2 + (2gc)*d + (-g c^2): the first two terms form a
    matmul over host-provided rows; the per-r constant is the per-partition
    bias of the Exp activation. fp32 matmul on this PE runs in slow
    LOW_HIGH emulation (~2.1us per 512-col op), so all matmuls use bf16:
      - expansion: d and d^2 are each split into 3 bf16 components (K=6).
        The coefficients -10 and 2r (integers < 256) are EXACT in bf16,
        so every product is exact; residual ~2e-4 in the exponent.
      - mm1/mm2: bf16 weights and activations (PE runs at the cold
        1.2 GHz clock here, ~0.83 ns/column; fp32 would double the MMs
        again for ~3e-4 accuracy we don't need against the ~2e-2 gate).
        f1 is stored bf16 after a range shift: f1' = softplus(x1) - kappa
        via Ln(e^-k * u1 + e^-k), which halves the bf16 absolute error;
        kappa is compensated in b2.
  * softplus(x) = ln(exp(x) + 1) via Exp then Ln activations (both live in
    the same ACT table set; no native softplus table is deployed).
  * Elements are flattened in (b, j, i) order so the final contraction
    over i is a native inner-axis vector reduce per 512-element chunk.
  * Channel dim is 64; two 512-element chunks are stacked to fill all 128
    partitions for mm1/softplus/mm2/softplus/mul/reduce.

Raw Bass (no Tile): the deployed walrus accepts at most one sync-wait per
instruction, so all cross-engine deps are standalone single-condition
wait_ge instructions; buffers are double-buffered with parity t % 2, and
same-engine dependent ACT ops are separated by an independent op so the
self-wait is nearly free.

Sharding: data-parallel over mb across 8 cores (4 molecules/core), params
replicated. No collectives; host splits inputs and reassembles outputs.
"""

import numpy as np

MB, ATOM, HD = 32, 64, 64
R = 128                     # effective RBF count (of 300)
GAMMA, RES = 10.0, 0.1
KAPPA = 0.875               # f1 range shift (exact in bf16)
NCORES = 8
MBC = MB // NCORES          # molecules per core
E = MBC * ATOM * ATOM       # flattened (b, j, i) elements per core
CH = 512                    # e-chunk (one PSUM bank col-width)
NCHUNK = E // CH
NPAIR = NCHUNK // 2

_CACHE = {}


def build_bass():
    from contextlib import ExitStack

    import concourse.bass as bass
    from concourse import mybir

    f32 = mybir.dt.float32
    bf16 = mybir.dt.bfloat16
    AF = mybir.ActivationFunctionType

    NM = NPAIR // 2  # macro-iterations of 2 pairs (4 chunks, 2048 elems)

    nc = bass.Bass()
    dd = nc.declare_dram_parameter("dd", [6, E], bf16, isOutput=False)
    coef = nc.declare_dram_parameter("coef", [6, R], bf16, isOutput=False)
    bexp = nc.declare_dram_parameter("bexp", [R, 1], f32, isOutput=False)
    w1 = nc.declare_dram_parameter("w1", [R, HD], bf16, isOutput=False)
    b1p = nc.declare_dram_parameter("b1p", [128, 1], f32, isOutput=False)
    w2 = nc.declare_dram_parameter("w2", [2 * HD, HD], bf16, isOutput=False)
    b2p = nc.declare_dram_parameter("b2p", [128, 1], f32, isOutput=False)
    hs = nc.declare_dram_parameter("hs", [128, MBC * ATOM], f32, isOutput=False)
    emk = nc.declare_dram_parameter("emk", [128, 1], f32, isOutput=False)
    res = nc.declare_dram_parameter("res", [128, NPAIR * 8], f32, isOutput=True)

    EMK = float(np.exp(-KAPPA))

    with ExitStack() as ctx:
        en = ctx.enter_context

        dd_sb = en(nc.sbuf_tensor("dd_sb", [6, E], bf16))
        coef_sb = en(nc.sbuf_tensor("coef_sb", [6, R], bf16))
        bexp_sb = en(nc.sbuf_tensor("bexp_sb", [R, 1], f32))
        w1_sb = en(nc.sbuf_tensor("w1_sb", [R, HD], bf16))
        b1p_sb = en(nc.sbuf_tensor("b1p_sb", [128, 1], f32))
        w2_sb = en(nc.sbuf_tensor("w2_sb", [2 * HD, HD], bf16))
        b2p_sb = en(nc.sbuf_tensor("b2p_sb", [128, 1], f32))
        hs_sb = en(nc.sbuf_tensor("hs_sb", [128, MBC * ATOM], f32))
        emk_sb = en(nc.sbuf_tensor("emk_sb", [128, 1], f32))
        res_sb = en(nc.sbuf_tensor("res_sb", [128, NPAIR * 8], f32))

        # per-pair rbf tiles (parity k%2); per-macro layer tiles
        rbf_sb = [en(nc.sbuf_tensor(f"rbf{i}", [128, 2 * CH], bf16)) for i in (0, 1)]
        u1_sb = en(nc.sbuf_tensor("u1_sb", [128, 2 * CH], f32))
        f1_sb = en(nc.sbuf_tensor("f1_sb", [128, 2 * CH], bf16))
        u2_sb = en(nc.sbuf_tensor("u2_sb", [128, 2 * CH], f32))
        f2_sb = [en(nc.sbuf_tensor(f"f2_{i}", [128, 2 * CH], f32)) for i in (0, 1)]
        prod_sb = [en(nc.sbuf_tensor(f"prod{i}", [128, 16, ATOM], f32)) for i in (0, 1)]

        exp_ps = [en(nc.psum_tensor(f"expps{i}", [128, 2 * CH], f32)) for i in (0, 1)]
        mm1_ps = en(nc.psum_tensor("mm1ps", [128, 2 * CH], f32))
        mm2_ps = en(nc.psum_tensor("mm2ps", [128, 2 * CH], f32))

        dma_sem = en(nc.semaphore("dma_sem"))
        pe_sem = en(nc.semaphore("pe_sem"))
        act_sem = en(nc.semaphore("act_sem"))
        dve_sem = en(nc.semaphore("dve_sem"))

        LOADS = 9  # input DMA transfers

        # ---- software-pipelined schedule (macro m = pairs 2m, 2m+1) ----
        # PE:  exp(0..1) | per m: exp(2m+2), exp(2m+3), mm1(m) x4, mm2(m) x4
        # ACT: rbf(0..1) | per m: u1(m), f2(m-1), f1(m), rbf(2m+2), u2(m),
        #                         rbf(2m+3) | f2(NM-1)
        def seq_counts(names):
            return {n: i + 1 for i, n in enumerate(names)}

        pe_ops = ["exp0a", "exp0b", "exp1a", "exp1b"]
        for m in range(NM):
            for k in (2 * m + 2, 2 * m + 3):
                if k < NPAIR:
                    pe_ops += [f"exp{k}a", f"exp{k}b"]
            pe_ops += [f"mm1_{m}_{i}" for i in range(4)]
            pe_ops += [f"mm2_{m}_{i}" for i in range(4)]
        PEC = seq_counts(pe_ops)

        act_ops = ["rbf0", "rbf1"]
        for m in range(NM):
            act_ops.append(f"u1_{m}")
            if m >= 1:
                act_ops.append(f"f2_{m - 1}")
            act_ops.append(f"f1_{m}")
            if 2 * m + 2 < NPAIR:
                act_ops.append(f"rbf{2 * m + 2}")
            act_ops.append(f"u2_{m}")
            if 2 * m + 3 < NPAIR:
                act_ops.append(f"rbf{2 * m + 3}")
        act_ops.append(f"f2_{NM - 1}")
        ACTC = seq_counts(act_ops)

        dve_ops = []
        for m in range(NM):
            dve_ops += [f"mul{m}", f"red{m}"]
        DVEC = seq_counts(dve_ops)

        with nc.Block() as block:

            @block.gpsimd
            def _(g):
                for dst, src_ in [
                    (dd_sb, dd), (coef_sb, coef), (bexp_sb, bexp),
                    (w1_sb, w1), (b1p_sb, b1p), (w2_sb, w2), (b2p_sb, b2p),
                    (hs_sb, hs), (emk_sb, emk),
                ]:
                    g.dma_start(dst[:], src_[:]).then_inc(dma_sem, 16)
                # output store after the last reduce
                g.wait_ge(dve_sem, DVEC[f"red{NM - 1}"])
                g.dma_start(res[:], res_sb[:]).then_inc(dma_sem, 16)
                g.wait_ge(dma_sem, 16 * (LOADS + 1))

            def emit_exp_mm(pe, k):
                p = k % 2
                for half in range(2):
                    q = 2 * k + half
                    pe.matmul(
                        exp_ps[p][:, half * CH : (half + 1) * CH],
                        coef_sb[:],
                        dd_sb[:, q * CH : (q + 1) * CH],
                        start=True, stop=True,
                    ).then_inc(pe_sem, 1)

            @block.tensor
            def _(pe):
                pe.wait_ge(dma_sem, 16 * LOADS)
                emit_exp_mm(pe, 0)
                emit_exp_mm(pe, 1)
                for m in range(NM):
                    for k in (2 * m + 2, 2 * m + 3):
                        if k < NPAIR:
                            # exp_ps[k%2] freed once ACT read rbf(k-2)
                            pe.wait_ge(act_sem, ACTC[f"rbf{k - 2}"])
                            emit_exp_mm(pe, k)
                    # mm1(m): pairs a=2m (cols 0:512), b=2m+1 (cols 512:1024)
                    pe.wait_ge(act_sem, ACTC[f"rbf{2 * m + 1}"])
                    pe.matmul(
                        mm1_ps[0:64, 0:CH], w1_sb[:], rbf_sb[0][:, 0:CH],
                        start=True, stop=True,
                    ).then_inc(pe_sem, 1)
                    pe.matmul(
                        mm1_ps[64:128, 0:CH], w1_sb[:], rbf_sb[0][:, CH : 2 * CH],
                        start=True, stop=True,
                    ).then_inc(pe_sem, 1)
                    pe.matmul(
                        mm1_ps[0:64, CH : 2 * CH], w1_sb[:], rbf_sb[1][:, 0:CH],
                        start=True, stop=True,
                    ).then_inc(pe_sem, 1)
                    pe.matmul(
                        mm1_ps[64:128, CH : 2 * CH], w1_sb[:], rbf_sb[1][:, CH : 2 * CH],
                        start=True, stop=True,
                    ).then_inc(pe_sem, 1)
                    # mm2(m): grouped by lhsT to minimize weight reloads
                    pe.wait_ge(act_sem, ACTC[f"f1_{m}"])
                    pe.matmul(
                        mm2_ps[0:64, 0:CH], w2_sb[0:64, :], f1_sb[0:64, 0:CH],
                        start=True, stop=True,
                    ).then_inc(pe_sem, 1)
                    pe.matmul(
                        mm2_ps[0:64, CH : 2 * CH], w2_sb[0:64, :], f1_sb[0:64, CH : 2 * CH],
                        start=True, stop=True,
                    ).then_inc(pe_sem, 1)
                    pe.matmul(
                        mm2_ps[64:128, 0:CH], w2_sb[64:128, :], f1_sb[64:128, 0:CH],
                        start=True, stop=True,
                    ).then_inc(pe_sem, 1)
                    pe.matmul(
                        mm2_ps[64:128, CH : 2 * CH], w2_sb[64:128, :],
                        f1_sb[64:128, CH : 2 * CH],
                        start=True, stop=True,
                    ).then_inc(pe_sem, 1)

            @block.scalar
            def _(act):
                act.wait_ge(dma_sem, 16 * LOADS)

                def rbf_act(k):
                    p = k % 2
                    act.wait_ge(pe_sem, PEC[f"exp{k}b"])
                    act.activation(
                        rbf_sb[p][:], exp_ps[p][:], AF.Exp, bias=bexp_sb[:]
                    ).then_inc(act_sem, 1)

                def f2_act(m):
                    p = m % 2
                    act.wait_ge(act_sem, ACTC[f"u2_{m}"])
                    if m >= 2:  # f2_sb[p] freed by DVE mul(m-2)
                        act.wait_ge(dve_sem, DVEC[f"mul{m - 2}"])
                    act.activation(
                        f2_sb[p][:], u2_sb[:], AF.Ln, bias=1.0
                    ).then_inc(act_sem, 1)

                rbf_act(0)
                rbf_act(1)
                for m in range(NM):
                    act.wait_ge(pe_sem, PEC[f"mm1_{m}_3"])
                    act.activation(
                        u1_sb[:], mm1_ps[:], AF.Exp, bias=b1p_sb[:]
                    ).then_inc(act_sem, 1)
                    if m >= 1:
                        f2_act(m - 1)
                    act.wait_ge(act_sem, ACTC[f"u1_{m}"])
                    # f1' = ln(e^-k u1 + e^-k) = softplus(x1) - kappa, bf16
                    act.activation(
                        f1_sb[:], u1_sb[:], AF.Ln, bias=emk_sb[:], scale=EMK
                    ).then_inc(act_sem, 1)
                    if 2 * m + 2 < NPAIR:
                        rbf_act(2 * m + 2)
                    act.wait_ge(pe_sem, PEC[f"mm2_{m}_3"])
                    act.activation(
                        u2_sb[:], mm2_ps[:], AF.Exp, bias=b2p_sb[:]
                    ).then_inc(act_sem, 1)
                    if 2 * m + 3 < NPAIR:
                        rbf_act(2 * m + 3)
                f2_act(NM - 1)

            @block.vector
            def _(ve):
                ve.wait_ge(dma_sem, 16 * LOADS)
                for m in range(NM):
                    p = m % 2
                    b = m // (NM // MBC)
                    ve.wait_ge(act_sem, ACTC[f"f2_{m}"])
                    if m >= 2:  # prod_sb[p] freed by red(m-2)
                        ve.wait_ge(dve_sem, DVEC[f"red{m - 2}"])
                    ve.tensor_mul(
                        prod_sb[p][:],
                        f2_sb[p][:].rearrange("p (j i) -> p j i", i=ATOM),
                        hs_sb[:, b * ATOM : (b + 1) * ATOM][:, None, :].broadcast_to(
                            [128, 16, ATOM]
                        ),
                    ).then_inc(dve_sem, 1)
                    ve.wait_ge(dve_sem, DVEC[f"mul{m}"])
                    ve.reduce_sum(
                        res_sb[:, m * 16 : (m + 1) * 16],
                        prod_sb[p][:],
                        axis=mybir.AxisListType.X,
                    ).then_inc(dve_sem, 1)

    return nc


def _split_bf16(x, n):
    """Split fp32 array into n bf16 components summing to ~x."""
    import ml_dtypes

    bf = ml_dtypes.bfloat16
    x = x.astype(np.float32)
    parts = []
    for _ in range(n):
        p = x.astype(bf)
        parts.append(p)
        x = x - p.astype(np.float32)
    return parts


def host_prep(h, dist, W1, b1, W2, b2):
    """Build per-core input maps (numpy only, layout/index prep)."""
    import ml_dtypes

    bf = ml_dtypes.bfloat16
    f4 = np.float32
    r = np.arange(R, dtype=f4)
    coef = np.stack(
        [np.full(R, -GAMMA, f4)] * 3 + [(2.0 * r).astype(f4)] * 3
    ).astype(bf)
    bexp = (-GAMMA * (RES * r) ** 2).astype(f4)[:, None]
    w1b = W1[:R].astype(f4).astype(bf)
    b1p = np.concatenate([b1, b1]).astype(f4)[:, None]
    w2b = W2.astype(f4).astype(bf)
    w2d = np.ascontiguousarray(np.concatenate([w2b, w2b], 0))
    # kappa compensation: out2 = W2dev.T @ (f1 - kappa) + b2 + kappa*colsum(W2dev)
    b2c = (b2 + KAPPA * w2b.astype(f4).sum(0)).astype(f4)
    b2p = np.concatenate([b2c, b2c]).astype(f4)[:, None]

    in_maps = []
    for g in range(NCORES):
        dist_c = dist[g * MBC : (g + 1) * MBC].astype(f4)
        dperm = np.ascontiguousarray(dist_c.transpose(0, 2, 1)).reshape(-1)  # (b,j,i)
        d2 = (dperm * dperm).astype(f4)
        ddv = np.ascontiguousarray(np.stack(_split_bf16(d2, 3) + _split_bf16(dperm, 3)))
        h_c = h[g * MBC : (g + 1) * MBC].astype(f4)
        ht = np.ascontiguousarray(h_c.transpose(2, 0, 1)).reshape(HD, MBC * ATOM)
        hsv = np.ascontiguousarray(np.concatenate([ht, ht], 0))
        in_maps.append(
            {
                "dd": ddv, "coef": coef, "bexp": bexp,
                "w1": w1b, "b1p": b1p, "w2": w2d, "b2p": b2p, "hs": hsv,
                "emk": np.full((128, 1), np.exp(-KAPPA), f4),
            }
        )
    return in_maps


def decode_res(res_np):
    """res [128, 128] -> out_core [MBC, ATOM(j), HD(c)].

    res[cc, t*8+jl]: b = t//4, sig = t%4, j = 16*sig + 8*(cc>=64) + jl,
    c = cc % 64.
    """
    r5 = res_np.reshape(2, HD, MBC, NPAIR // MBC, 8)  # [half, c, b, sig, jl]
    return np.ascontiguousarray(r5.transpose(2, 3, 0, 4, 1)).reshape(MBC, ATOM, HD)


def kernel(h, dist, W1, b1, W2, b2):
    from concourse.bass_utils import run_bass_kernel_spmd

    if "nc" not in _CACHE:
        _CACHE["nc"] = build_bass()
    nc = _CACHE["nc"]
    in_maps = host_prep(h, dist, W1, b1, W2, b2)
    out = run_bass_kernel_spmd(nc, in_maps, list(range(NCORES)))
    cores = [decode_res(out.results[g]["res"]) for g in range(NCORES)]
    return np.concatenate(cores, axis=0).astype(np.float32)


# revision 21
# speedup vs baseline: 2.2244x; 1.2217x over previous
"""CFConv (SchNet continuous-filter convolution) Trainium2 kernel.

Math (per molecule b):
    rbf[b,i,j,r] = exp(-gamma * (dist[b,i,j] - r*res)^2),  r = 0..299
    f = softplus(rbf @ W1 + b1); f = softplus(f @ W2 + b2)
    out[b,j,c] = sum_i h[b,i,c] * f[b,i,j,c]

Device-level reformulations:
  * dist < 10.0 and gamma=10 => centers r >= 128 (c_r >= 12.8) give
    exp(<= -78) ~ 1e-34: identically negligible in fp32. K: 300 -> 128.
  * -g(d-c)^2 = (-g)*d^2 + (2gc)*d + (-g c^2): the first two terms form a
    matmul over host-provided rows; the per-r constant is the per-partition
    bias of the Exp activation. fp32 matmul on this PE runs in slow
    LOW_HIGH emulation (~2.1us per 512-col op), so all matmuls use bf16:
      - expansion: d and d^2 are each split into 3 bf16 components (K=6).
        The coefficients -10 and 2r (integers < 256) are EXACT in bf16,
        so every product is exact; residual ~2e-4 in the exponent.
      - mm1/mm2: bf16 weights and activations (PE runs at the cold
        1.2 GHz clock here, ~0.83 ns/column; fp32 would double the MMs
        again for ~3e-4 accuracy we don't need against the ~2e-2 gate).
        f1 is stored bf16 after a range shift: f1' = softplus(x1) - kappa
        via Ln(e^-k * u1 + e^-k), which halves the bf16 absolute error;
        kappa is compensated in b2.
  * softplus(x) = ln(exp(x) + 1) via Exp then Ln activations (both live in
    the same ACT table set; no native softplus table is deployed).
  * Elements are flattened in (b, j, i) order so the final contraction
    over i is a native inner-axis vector reduce per 512-element chunk.
  * Channel dim is 64; two 512-element chunks are stacked to fill all 128
    partitions for mm1/softplus/mm2/softplus/mul/reduce.

Raw Bass (no Tile): the deployed walrus accepts at most one sync-wait per
instruction, so all cross-engine deps are standalone single-condition
wait_ge instructions; buffers are double-buffered with parity t % 2, and
same-engine dependent ACT ops are separated by an independent op so the
self-wait is nearly free.

Sharding: data-parallel over mb across 8 cores (4 molecules/core), params
replicated. No collectives; host splits inputs and reassembles outputs.
"""

import numpy as np

MB, ATOM, HD = 32, 64, 64
R = 128                     # effective RBF count (of 300)
GAMMA, RES = 10.0, 0.1
KAPPA = 0.875               # f1 range shift (exact in bf16)
NCORES = 8
MBC = MB // NCORES          # molecules per core
E = MBC * ATOM * ATOM       # flattened (b, j, i) elements per core
CH = 512                    # e-chunk (one PSUM bank col-width)
NCHUNK = E // CH
NPAIR = NCHUNK // 2

_CACHE = {}


def build_bass():
    from contextlib import ExitStack

    import concourse.bass as bass
    from concourse import mybir

    f32 = mybir.dt.float32
    bf16 = mybir.dt.bfloat16
    AF = mybir.ActivationFunctionType

    NM = NPAIR // 2  # macro-iterations of 2 pairs (4 chunks, 2048 elems)

    nc = bass.Bass()
    dd = nc.declare_dram_parameter("dd", [6, E], bf16, isOutput=False)
    coef = nc.declare_dram_parameter("coef", [6, R], bf16, isOutput=False)
    bexp = nc.declare_dram_parameter("bexp", [R, 1], f32, isOutput=False)
    w1 = nc.declare_dram_parameter("w1", [R, HD], bf16, isOutput=False)
    b1p = nc.declare_dram_parameter("b1p", [128, 1], f32, isOutput=False)
    w2 = nc.declare_dram_parameter("w2", [2 * HD, HD], bf16, isOutput=False)
    b2p = nc.declare_dram_parameter("b2p", [128, 1], f32, isOutput=False)
    hs = nc.declare_dram_parameter("hs", [128, MBC * ATOM], f32, isOutput=False)
    emk = nc.declare_dram_parameter("emk", [128, 1], f32, isOutput=False)
    res = nc.declare_dram_parameter("res", [128, NPAIR * 8], f32, isOutput=True)

    EMK = float(np.exp(-KAPPA))

    with ExitStack() as ctx:
        en = ctx.enter_context

        dd_sb = en(nc.sbuf_tensor("dd_sb", [6, E], bf16))
        coef_sb = en(nc.sbuf_tensor("coef_sb", [6, R], bf16))
        bexp_sb = en(nc.sbuf_tensor("bexp_sb", [R, 1], f32))
        w1_sb = en(nc.sbuf_tensor("w1_sb", [R, HD], bf16))
        b1p_sb = en(nc.sbuf_tensor("b1p_sb", [128, 1], f32))
        w2_sb = en(nc.sbuf_tensor("w2_sb", [2 * HD, HD], bf16))
        b2p_sb = en(nc.sbuf_tensor("b2p_sb", [128, 1], f32))
        hs_sb = en(nc.sbuf_tensor("hs_sb", [128, MBC * ATOM], f32))
        emk_sb = en(nc.sbuf_tensor("emk_sb", [128, 1], f32))
        res_sb = en(nc.sbuf_tensor("res_sb", [128, NPAIR * 8], f32))

        # per-pair rbf tiles (parity k%2); per-macro layer tiles
        rbf_sb = [en(nc.sbuf_tensor(f"rbf{i}", [128, 2 * CH], bf16)) for i in (0, 1)]
        u1_sb = en(nc.sbuf_tensor("u1_sb", [128, 2 * CH], f32))
        f1_sb = en(nc.sbuf_tensor("f1_sb", [128, 2 * CH], bf16))
        u2_sb = en(nc.sbuf_tensor("u2_sb", [128, 2 * CH], f32))
        f2_sb = [en(nc.sbuf_tensor(f"f2_{i}", [128, 2 * CH], f32)) for i in (0, 1)]
        prod_sb = [en(nc.sbuf_tensor(f"prod{i}", [128, 16, ATOM], f32)) for i in (0, 1)]

        exp_ps = [en(nc.psum_tensor(f"expps{i}", [128, 2 * CH], f32)) for i in (0, 1)]
        mm1_ps = [en(nc.psum_tensor(f"mm1ps{i}", [128, CH], f32)) for i in (0, 1)]
        mm2_ps = [en(nc.psum_tensor(f"mm2ps{i}", [128, CH], f32)) for i in (0, 1)]

        dma_sem = en(nc.semaphore("dma_sem"))
        pe_sem = en(nc.semaphore("pe_sem"))
        act_sem = en(nc.semaphore("act_sem"))
        dve_sem = en(nc.semaphore("dve_sem"))

        LOADS = 9  # input DMA transfers

        # ---- software-pipelined schedule (macro m = pairs 2m, 2m+1) ----
        # PE:  exp(0..3), mm1(0), mm1(1)
        #      | per m: exp(2m+4), exp(2m+5), mm1(2m+2), mm1(2m+3),
        #               mm2(2m), mm2(2m+1)
        # ACT: rbf(0), rbf(1)
        #      | per m: rbf(2m+2), rbf(2m+3), u1(2m), u1(2m+1),
        #               u2(2m-2), u2(2m-1), f1(m), f2(m-1)
        #      | u2(2NM-2), u2(2NM-1), f2(NM-1)
        # mm1 runs one macro ahead of f1/mm2; u2/f2 lag one macro, so the
        # mm1->u1->f1->mm2 chain of macro m overlaps macro m+1's mm1.
        def seq_counts(names):
            return {n: i + 1 for i, n in enumerate(names)}

        pe_ops = ["exp0a", "exp0b", "exp1a", "exp1b", "exp2a", "exp2b",
                  "exp3a", "exp3b", "mm1_0a", "mm1_0b", "mm1_1a", "mm1_1b"]
        for m in range(NM):
            for k in (2 * m + 4, 2 * m + 5):
                if k < NPAIR:
                    pe_ops += [f"exp{k}a", f"exp{k}b"]
            for k in (2 * m + 2, 2 * m + 3):
                if k < NPAIR:
                    pe_ops += [f"mm1_{k}a", f"mm1_{k}b"]
            pe_ops += [f"mm2_{2 * m}a", f"mm2_{2 * m}b",
                       f"mm2_{2 * m + 1}a", f"mm2_{2 * m + 1}b"]
        PEC = seq_counts(pe_ops)

        act_ops = ["rbf0", "rbf1"]
        for m in range(NM):
            for k in (2 * m + 2, 2 * m + 3):
                if k < NPAIR:
                    act_ops.append(f"rbf{k}")
            act_ops += [f"u1_{2 * m}", f"u1_{2 * m + 1}"]
            if m >= 1:
                act_ops += [f"u2_{2 * m - 2}", f"u2_{2 * m - 1}"]
            act_ops.append(f"f1_{m}")
            if m >= 1:
                act_ops.append(f"f2_{m - 1}")
        act_ops += [f"u2_{2 * NM - 2}", f"u2_{2 * NM - 1}", f"f2_{NM - 1}"]
        ACTC = seq_counts(act_ops)

        dve_ops = []
        for m in range(NM):
            dve_ops += [f"mul{m}", f"red{m}"]
        DVEC = seq_counts(dve_ops)

        with nc.Block() as block:

            @block.gpsimd
            def _(g):
                for dst, src_ in [
                    (dd_sb, dd), (coef_sb, coef), (bexp_sb, bexp),
                    (w1_sb, w1), (b1p_sb, b1p), (w2_sb, w2), (b2p_sb, b2p),
                    (hs_sb, hs), (emk_sb, emk),
                ]:
                    g.dma_start(dst[:], src_[:]).then_inc(dma_sem, 16)
                # output store after the last reduce
                g.wait_ge(dve_sem, DVEC[f"red{NM - 1}"])
                g.dma_start(res[:], res_sb[:]).then_inc(dma_sem, 16)
                g.wait_ge(dma_sem, 16 * (LOADS + 1))

            def emit_exp_mm(pe, k):
                p = k % 2
                for half in range(2):
                    q = 2 * k + half
                    pe.matmul(
                        exp_ps[p][:, half * CH : (half + 1) * CH],
                        coef_sb[:],
                        dd_sb[:, q * CH : (q + 1) * CH],
                        start=True, stop=True,
                    ).then_inc(pe_sem, 1)

            def emit_mm1(pe, k):
                p = k % 2
                pe.matmul(
                    mm1_ps[p][0:64, :], w1_sb[:], rbf_sb[p][:, 0:CH],
                    start=True, stop=True,
                ).then_inc(pe_sem, 1)
                pe.matmul(
                    mm1_ps[p][64:128, :], w1_sb[:], rbf_sb[p][:, CH : 2 * CH],
                    start=True, stop=True,
                ).then_inc(pe_sem, 1)

            def emit_mm2(pe, k):
                p = k % 2
                pe.matmul(
                    mm2_ps[p][0:64, :], w2_sb[0:64, :],
                    f1_sb[0:64, p * CH : (p + 1) * CH],
                    start=True, stop=True,
                ).then_inc(pe_sem, 1)
                pe.matmul(
                    mm2_ps[p][64:128, :], w2_sb[64:128, :],
                    f1_sb[64:128, p * CH : (p + 1) * CH],
                    start=True, stop=True,
                ).then_inc(pe_sem, 1)

            @block.tensor
            def _(pe):
                pe.wait_ge(dma_sem, 16 * LOADS)
                emit_exp_mm(pe, 0)
                emit_exp_mm(pe, 1)
                pe.wait_ge(act_sem, ACTC["rbf0"])
                emit_exp_mm(pe, 2)
                pe.wait_ge(act_sem, ACTC["rbf1"])
                emit_exp_mm(pe, 3)
                emit_mm1(pe, 0)  # rbf0/rbf1 waits subsumed above
                emit_mm1(pe, 1)
                for m in range(NM):
                    for k in (2 * m + 4, 2 * m + 5):
                        if k < NPAIR:
                            pe.wait_ge(act_sem, ACTC[f"rbf{k - 2}"])
                            emit_exp_mm(pe, k)
                    for k in (2 * m + 2, 2 * m + 3):
                        if k < NPAIR:
                            pe.wait_ge(act_sem, ACTC[f"u1_{k - 2}"])
                            emit_mm1(pe, k)
                    pe.wait_ge(act_sem, ACTC[f"f1_{m}"])
                    emit_mm2(pe, 2 * m)
                    emit_mm2(pe, 2 * m + 1)

            @block.scalar
            def _(act):
                act.wait_ge(dma_sem, 16 * LOADS)

                def rbf_act(k, wait_mm1=True):
                    p = k % 2
                    if wait_mm1:
                        act.wait_ge(pe_sem, PEC[f"mm1_{k - 2}b"])
                    else:
                        act.wait_ge(pe_sem, PEC[f"exp{k}b"])
                    act.activation(
                        rbf_sb[p][:], exp_ps[p][:], AF.Exp, bias=bexp_sb[:]
                    ).then_inc(act_sem, 1)

                rbf_act(0, wait_mm1=False)
                rbf_act(1, wait_mm1=False)
                for m in range(NM):
                    for k in (2 * m + 2, 2 * m + 3):
                        if k < NPAIR:
                            rbf_act(k)
                    if m >= 1:  # u1_sb WAR vs f1(m-1) read (same engine)
                        act.wait_ge(act_sem, ACTC[f"f1_{m - 1}"])
                    for k in (2 * m, 2 * m + 1):
                        if 2 * m + 3 >= NPAIR:
                            # rbf waits absent near the tail: wait mm1 directly
                            act.wait_ge(pe_sem, PEC[f"mm1_{k}b"])
                        act.activation(
                            u1_sb[:, (k % 2) * CH : (k % 2 + 1) * CH],
                            mm1_ps[k % 2][:], AF.Exp, bias=b1p_sb[:],
                        ).then_inc(act_sem, 1)
                    if m >= 1:
                        act.wait_ge(pe_sem, PEC[f"mm2_{2 * m - 1}b"])
                        if m >= 2:  # u2_sb WAR vs f2(m-2) read (same engine)
                            act.wait_ge(act_sem, ACTC[f"f2_{m - 2}"])
                        for k in (2 * m - 2, 2 * m - 1):
                            act.activation(
                                u2_sb[:, (k % 2) * CH : (k % 2 + 1) * CH],
                                mm2_ps[k % 2][:], AF.Exp, bias=b2p_sb[:],
                            ).then_inc(act_sem, 1)
                    # f1' = ln(e^-k u1 + e^-k) = softplus(x1) - kappa, bf16
                    act.wait_ge(act_sem, ACTC[f"u1_{2 * m + 1}"])
                    act.activation(
                        f1_sb[:], u1_sb[:], AF.Ln, bias=emk_sb[:], scale=EMK
                    ).then_inc(act_sem, 1)
                    if m >= 1:
                        act.wait_ge(act_sem, ACTC[f"u2_{2 * m - 1}"])
                        if m >= 3:  # f2_sb[(m-1)%2] freed by DVE mul(m-3)
                            act.wait_ge(dve_sem, DVEC[f"mul{m - 3}"])
                        act.activation(
                            f2_sb[(m - 1) % 2][:], u2_sb[:], AF.Ln, bias=1.0
                        ).then_inc(act_sem, 1)
                # epilogue: u2 for the last two pairs, then f2(NM-1)
                act.wait_ge(pe_sem, PEC[f"mm2_{2 * NM - 1}b"])
                act.wait_ge(act_sem, ACTC[f"f2_{NM - 2}"])
                for k in (2 * NM - 2, 2 * NM - 1):
                    act.activation(
                        u2_sb[:, (k % 2) * CH : (k % 2 + 1) * CH],
                        mm2_ps[k % 2][:], AF.Exp, bias=b2p_sb[:],
                    ).then_inc(act_sem, 1)
                act.wait_ge(act_sem, ACTC[f"u2_{2 * NM - 1}"])
                act.wait_ge(dve_sem, DVEC[f"mul{NM - 3}"])
                act.activation(
                    f2_sb[(NM - 1) % 2][:], u2_sb[:], AF.Ln, bias=1.0
                ).then_inc(act_sem, 1)

            @block.vector
            def _(ve):
                ve.wait_ge(dma_sem, 16 * LOADS)
                for m in range(NM):
                    p = m % 2
                    b = m // (NM // MBC)
                    ve.wait_ge(act_sem, ACTC[f"f2_{m}"])
                    if m >= 2:  # prod_sb[p] freed by red(m-2)
                        ve.wait_ge(dve_sem, DVEC[f"red{m - 2}"])
                    ve.tensor_mul(
                        prod_sb[p][:],
                        f2_sb[p][:].rearrange("p (j i) -> p j i", i=ATOM),
                        hs_sb[:, b * ATOM : (b + 1) * ATOM][:, None, :].broadcast_to(
                            [128, 16, ATOM]
                        ),
                    ).then_inc(dve_sem, 1)
                    ve.wait_ge(dve_sem, DVEC[f"mul{m}"])
                    ve.reduce_sum(
                        res_sb[:, m * 16 : (m + 1) * 16],
                        prod_sb[p][:],
                        axis=mybir.AxisListType.X,
                    ).then_inc(dve_sem, 1)

    return nc


def _split_bf16(x, n):
    """Split fp32 array into n bf16 components summing to ~x."""
    import ml_dtypes

    bf = ml_dtypes.bfloat16
    x = x.astype(np.float32)
    parts = []
    for _ in range(n):
        p = x.astype(bf)
        parts.append(p)
        x = x - p.astype(np.float32)
    return parts


def host_prep(h, dist, W1, b1, W2, b2):
    """Build per-core input maps (numpy only, layout/index prep)."""
    import ml_dtypes

    bf = ml_dtypes.bfloat16
    f4 = np.float32
    r = np.arange(R, dtype=f4)
    coef = np.stack(
        [np.full(R, -GAMMA, f4)] * 3 + [(2.0 * r).astype(f4)] * 3
    ).astype(bf)
    bexp = (-GAMMA * (RES * r) ** 2).astype(f4)[:, None]
    w1b = W1[:R].astype(f4).astype(bf)
    b1p = np.concatenate([b1, b1]).astype(f4)[:, None]
    w2b = W2.astype(f4).astype(bf)
    w2d = np.ascontiguousarray(np.concatenate([w2b, w2b], 0))
    # kappa compensation: out2 = W2dev.T @ (f1 - kappa) + b2 + kappa*colsum(W2dev)
    b2c = (b2 + KAPPA * w2b.astype(f4).sum(0)).astype(f4)
    b2p = np.concatenate([b2c, b2c]).astype(f4)[:, None]

    in_maps = []
    for g in range(NCORES):
        dist_c = dist[g * MBC : (g + 1) * MBC].astype(f4)
        dperm = np.ascontiguousarray(dist_c.transpose(0, 2, 1)).reshape(-1)  # (b,j,i)
        d2 = (dperm * dperm).astype(f4)
        ddv = np.ascontiguousarray(np.stack(_split_bf16(d2, 3) + _split_bf16(dperm, 3)))
        h_c = h[g * MBC : (g + 1) * MBC].astype(f4)
        ht = np.ascontiguousarray(h_c.transpose(2, 0, 1)).reshape(HD, MBC * ATOM)
        hsv = np.ascontiguousarray(np.concatenate([ht, ht], 0))
        in_maps.append(
            {
                "dd": ddv, "coef": coef, "bexp": bexp,
                "w1": w1b, "b1p": b1p, "w2": w2d, "b2p": b2p, "hs": hsv,
                "emk": np.full((128, 1), np.exp(-KAPPA), f4),
            }
        )
    return in_maps


def decode_res(res_np):
    """res [128, 128] -> out_core [MBC, ATOM(j), HD(c)].

    res[cc, t*8+jl]: b = t//4, sig = t%4, j = 16*sig + 8*(cc>=64) + jl,
    c = cc % 64.
    """
    r5 = res_np.reshape(2, HD, MBC, NPAIR // MBC, 8)  # [half, c, b, sig, jl]
    return np.ascontiguousarray(r5.transpose(2, 3, 0, 4, 1)).reshape(MBC, ATOM, HD)


def kernel(h, dist, W1, b1, W2, b2):
    from concourse.bass_utils import run_bass_kernel_spmd

    if "nc" not in _CACHE:
        _CACHE["nc"] = build_bass()
    nc = _CACHE["nc"]
    in_maps = host_prep(h, dist, W1, b1, W2, b2)
    out = run_bass_kernel_spmd(nc, in_maps, list(range(NCORES)))
    cores = [decode_res(out.results[g]["res"]) for g in range(NCORES)]
    return np.concatenate(cores, axis=0).astype(np.float32)


# revision 23
# speedup vs baseline: 2.2514x; 1.0122x over previous
"""CFConv (SchNet continuous-filter convolution) Trainium2 kernel.

Math (per molecule b):
    rbf[b,i,j,r] = exp(-gamma * (dist[b,i,j] - r*res)^2),  r = 0..299
    f = softplus(rbf @ W1 + b1); f = softplus(f @ W2 + b2)
    out[b,j,c] = sum_i h[b,i,c] * f[b,i,j,c]

Device-level reformulations:
  * dist < 10.0 and gamma=10 => centers r >= 128 (c_r >= 12.8) give
    exp(<= -78) ~ 1e-34: identically negligible in fp32. K: 300 -> 128.
  * -g(d-c)^2 = (-g)*d^2 + (2gc)*d + (-g c^2): the first two terms form a
    matmul over host-provided rows; the per-r constant is the per-partition
    bias of the Exp activation. fp32 matmul on this PE runs in slow
    LOW_HIGH emulation (~2.1us per 512-col op), so all matmuls use bf16:
      - expansion: d and d^2 are each split into 3 bf16 components (K=6).
        The coefficients -10 and 2r (integers < 256) are EXACT in bf16,
        so every product is exact; residual ~2e-4 in the exponent.
      - mm1/mm2: bf16 weights and activations (PE runs at the cold
        1.2 GHz clock here, ~0.83 ns/column; fp32 would double the MMs
        again for ~3e-4 accuracy we don't need against the ~2e-2 gate).
        f1 is stored bf16 after a range shift: f1' = softplus(x1) - kappa
        via Ln(e^-k * u1 + e^-k), which halves the bf16 absolute error;
        kappa is compensated in b2.
  * softplus(x) = ln(exp(x) + 1) via Exp then Ln activations (both live in
    the same ACT table set; no native softplus table is deployed).
  * Elements are flattened in (b, j, i) order so the final contraction
    over i is a native inner-axis vector reduce per 512-element chunk.
  * Channel dim is 64; two 512-element chunks are stacked to fill all 128
    partitions for mm1/softplus/mm2/softplus/mul/reduce.

Raw Bass (no Tile): the deployed walrus accepts at most one sync-wait per
instruction, so all cross-engine deps are standalone single-condition
wait_ge instructions; buffers are double-buffered with parity t % 2, and
same-engine dependent ACT ops are separated by an independent op so the
self-wait is nearly free.

Sharding: data-parallel over mb across 8 cores (4 molecules/core), params
replicated. No collectives; host splits inputs and reassembles outputs.
"""

import numpy as np

MB, ATOM, HD = 32, 64, 64
R = 128                     # effective RBF count (of 300)
GAMMA, RES = 10.0, 0.1
KAPPA = 0.875               # f1 range shift (exact in bf16)
NCORES = 8
MBC = MB // NCORES          # molecules per core
E = MBC * ATOM * ATOM       # flattened (b, j, i) elements per core
CH = 512                    # e-chunk (one PSUM bank col-width)
NCHUNK = E // CH
NPAIR = NCHUNK // 2

_CACHE = {}


def build_bass():
    from contextlib import ExitStack

    import concourse.bass as bass
    from concourse import mybir

    f32 = mybir.dt.float32
    bf16 = mybir.dt.bfloat16
    AF = mybir.ActivationFunctionType

    NM = NPAIR // 2  # macro-iterations of 2 pairs (4 chunks, 2048 elems)

    nc = bass.Bass()
    dd = nc.declare_dram_parameter("dd", [6, E], bf16, isOutput=False)
    coef = nc.declare_dram_parameter("coef", [6, R], bf16, isOutput=False)
    bexp = nc.declare_dram_parameter("bexp", [R, 1], f32, isOutput=False)
    w1 = nc.declare_dram_parameter("w1", [R, HD], bf16, isOutput=False)
    b1p = nc.declare_dram_parameter("b1p", [128, 1], f32, isOutput=False)
    w2 = nc.declare_dram_parameter("w2", [2 * HD, HD], bf16, isOutput=False)
    b2p = nc.declare_dram_parameter("b2p", [128, 1], f32, isOutput=False)
    hs = nc.declare_dram_parameter("hs", [128, MBC * ATOM], f32, isOutput=False)
    emk = nc.declare_dram_parameter("emk", [128, 1], f32, isOutput=False)
    res = nc.declare_dram_parameter("res", [128, NPAIR * 8], f32, isOutput=True)

    EMK = float(np.exp(-KAPPA))

    with ExitStack() as ctx:
        en = ctx.enter_context

        dd_sb = en(nc.sbuf_tensor("dd_sb", [6, E], bf16))
        coef_sb = en(nc.sbuf_tensor("coef_sb", [6, R], bf16))
        bexp_sb = en(nc.sbuf_tensor("bexp_sb", [R, 1], f32))
        w1_sb = en(nc.sbuf_tensor("w1_sb", [R, HD], bf16))
        b1p_sb = en(nc.sbuf_tensor("b1p_sb", [128, 1], f32))
        w2_sb = en(nc.sbuf_tensor("w2_sb", [2 * HD, HD], bf16))
        b2p_sb = en(nc.sbuf_tensor("b2p_sb", [128, 1], f32))
        hs_sb = en(nc.sbuf_tensor("hs_sb", [128, MBC * ATOM], f32))
        emk_sb = en(nc.sbuf_tensor("emk_sb", [128, 1], f32))
        res_sb = en(nc.sbuf_tensor("res_sb", [128, NPAIR * 8], f32))

        # per-pair rbf tiles (parity k%2); per-macro layer tiles
        rbf_sb = [en(nc.sbuf_tensor(f"rbf{i}", [128, 2 * CH], bf16)) for i in (0, 1)]
        u1_sb = en(nc.sbuf_tensor("u1_sb", [128, 2 * CH], f32))
        f1_sb = en(nc.sbuf_tensor("f1_sb", [128, 2 * CH], bf16))
        u2_sb = en(nc.sbuf_tensor("u2_sb", [128, 2 * CH], f32))
        f2_sb = [en(nc.sbuf_tensor(f"f2_{i}", [128, 2 * CH], f32)) for i in (0, 1)]
        prod_sb = [en(nc.sbuf_tensor(f"prod{i}", [128, 16, ATOM], f32)) for i in (0, 1)]

        exp_ps = [en(nc.psum_tensor(f"expps{i}", [128, 2 * CH], f32)) for i in (0, 1)]
        mm1_ps = [en(nc.psum_tensor(f"mm1ps{i}", [128, CH], f32)) for i in (0, 1)]
        mm2_ps = [en(nc.psum_tensor(f"mm2ps{i}", [128, CH], f32)) for i in (0, 1)]

        dma_sem = en(nc.semaphore("dma_sem"))
        pe_sem = en(nc.semaphore("pe_sem"))
        act_sem = en(nc.semaphore("act_sem"))
        dve_sem = en(nc.semaphore("dve_sem"))

        LOADS = 9  # input DMA transfers

        # ---- software-pipelined schedule (macro m = pairs 2m, 2m+1) ----
        # PE:  exp(0..3), mm1(0), mm1(1)
        #      | per m: exp(2m+4), exp(2m+5), mm1(2m+2), mm1(2m+3),
        #               mm2(2m), mm2(2m+1)
        # ACT: rbf(0), rbf(1)
        #      | per m: rbf(2m+2), rbf(2m+3), u1(2m), u1(2m+1),
        #               u2(2m-2), u2(2m-1), f1(m), f2(m-1)
        #      | u2(2NM-2), u2(2NM-1), f2(NM-1)
        # mm1 runs one macro ahead of f1/mm2; u2/f2 lag one macro, so the
        # mm1->u1->f1->mm2 chain of macro m overlaps macro m+1's mm1.
        def seq_counts(names):
            return {n: i + 1 for i, n in enumerate(names)}

        pe_ops = ["exp0a", "exp0b", "exp1a", "exp1b", "exp2a", "exp2b",
                  "exp3a", "exp3b", "mm1_0a", "mm1_0b", "mm1_1a", "mm1_1b"]
        for m in range(NM):
            for k in (2 * m + 4, 2 * m + 5):
                if k < NPAIR:
                    pe_ops += [f"exp{k}a", f"exp{k}b"]
            for k in (2 * m + 2, 2 * m + 3):
                if k < NPAIR:
                    pe_ops += [f"mm1_{k}a", f"mm1_{k}b"]
            pe_ops += [f"mm2_{2 * m}a", f"mm2_{2 * m}b",
                       f"mm2_{2 * m + 1}a", f"mm2_{2 * m + 1}b"]
        PEC = seq_counts(pe_ops)

        act_ops = ["rbf0", "rbf1"]
        for m in range(NM):
            for k in (2 * m + 2, 2 * m + 3):
                if k < NPAIR:
                    act_ops.append(f"rbf{k}")
            act_ops += [f"u1_{2 * m}", f"u1_{2 * m + 1}"]
            if m >= 1:
                act_ops += [f"u2_{2 * m - 2}", f"u2_{2 * m - 1}"]
            act_ops.append(f"f1_{m}")
            if m >= 1:
                act_ops.append(f"f2_{m - 1}")
        act_ops += [f"u2_{2 * NM - 2}", f"u2_{2 * NM - 1}", f"f2_{NM - 1}"]
        ACTC = seq_counts(act_ops)

        dve_ops = []
        for m in range(NM):
            dve_ops += [f"mul{m}", f"red{m}"]
        DVEC = seq_counts(dve_ops)

        with nc.Block() as block:

            @block.gpsimd
            def _(g):
                # staggered load batches; the issuing engine serializes at
                # batch boundaries so the counts are stable wait points
                for dst, src_ in [(coef_sb, coef), (dd_sb, dd), (bexp_sb, bexp)]:
                    g.dma_start(dst[:], src_[:]).then_inc(dma_sem, 16)
                g.wait_ge(dma_sem, 48)
                for dst, src_ in [(w1_sb, w1), (b1p_sb, b1p), (w2_sb, w2),
                                  (b2p_sb, b2p), (emk_sb, emk)]:
                    g.dma_start(dst[:], src_[:]).then_inc(dma_sem, 16)
                g.wait_ge(dma_sem, 128)
                g.dma_start(hs_sb[:], hs[:]).then_inc(dma_sem, 16)
                # output store after the last reduce
                g.wait_ge(dve_sem, DVEC[f"red{NM - 1}"])
                g.dma_start(res[:], res_sb[:]).then_inc(dma_sem, 16)
                g.wait_ge(dma_sem, 16 * (LOADS + 1))

            def emit_exp_mm(pe, k):
                p = k % 2
                for half in range(2):
                    q = 2 * k + half
                    pe.matmul(
                        exp_ps[p][:, half * CH : (half + 1) * CH],
                        coef_sb[:],
                        dd_sb[:, q * CH : (q + 1) * CH],
                        start=True, stop=True,
                    ).then_inc(pe_sem, 1)

            def emit_mm1(pe, k):
                p = k % 2
                pe.matmul(
                    mm1_ps[p][0:64, :], w1_sb[:], rbf_sb[p][:, 0:CH],
                    start=True, stop=True,
                ).then_inc(pe_sem, 1)
                pe.matmul(
                    mm1_ps[p][64:128, :], w1_sb[:], rbf_sb[p][:, CH : 2 * CH],
                    start=True, stop=True,
                ).then_inc(pe_sem, 1)

            def emit_mm2(pe, k):
                p = k % 2
                pe.matmul(
                    mm2_ps[p][0:64, :], w2_sb[0:64, :],
                    f1_sb[0:64, p * CH : (p + 1) * CH],
                    start=True, stop=True,
                ).then_inc(pe_sem, 1)
                pe.matmul(
                    mm2_ps[p][64:128, :], w2_sb[64:128, :],
                    f1_sb[64:128, p * CH : (p + 1) * CH],
                    start=True, stop=True,
                ).then_inc(pe_sem, 1)

            @block.tensor
            def _(pe):
                pe.wait_ge(dma_sem, 48)   # batch 1: coef, dd, bexp
                emit_exp_mm(pe, 0)
                emit_exp_mm(pe, 1)
                pe.wait_ge(act_sem, ACTC["rbf0"])
                emit_exp_mm(pe, 2)
                pe.wait_ge(act_sem, ACTC["rbf1"])
                emit_exp_mm(pe, 3)
                pe.wait_ge(dma_sem, 128)  # batch 2: weights/biases
                emit_mm1(pe, 0)  # rbf0/rbf1 waits subsumed above
                emit_mm1(pe, 1)
                for m in range(NM):
                    for k in (2 * m + 4, 2 * m + 5):
                        if k < NPAIR:
                            pe.wait_ge(act_sem, ACTC[f"rbf{k - 2}"])
                            emit_exp_mm(pe, k)
                    for k in (2 * m + 2, 2 * m + 3):
                        if k < NPAIR:
                            pe.wait_ge(act_sem, ACTC[f"u1_{k - 2}"])
                            emit_mm1(pe, k)
                    pe.wait_ge(act_sem, ACTC[f"f1_{m}"])
                    emit_mm2(pe, 2 * m)
                    emit_mm2(pe, 2 * m + 1)

            @block.scalar
            def _(act):
                act.wait_ge(dma_sem, 48)   # batch 1: bexp
                first_u1 = [True]
                first_f1 = [True]

                def rbf_act(k, wait_mm1=True):
                    p = k % 2
                    if wait_mm1:
                        act.wait_ge(pe_sem, PEC[f"mm1_{k - 2}b"])
                    else:
                        act.wait_ge(pe_sem, PEC[f"exp{k}b"])
                    act.activation(
                        rbf_sb[p][:], exp_ps[p][:], AF.Exp, bias=bexp_sb[:]
                    ).then_inc(act_sem, 1)

                rbf_act(0, wait_mm1=False)
                rbf_act(1, wait_mm1=False)
                for m in range(NM):
                    for k in (2 * m + 2, 2 * m + 3):
                        if k < NPAIR:
                            rbf_act(k)
                    if m >= 1:  # u1_sb WAR vs f1(m-1) read (same engine)
                        act.wait_ge(act_sem, ACTC[f"f1_{m - 1}"])
                    if first_u1[0]:
                        act.wait_ge(dma_sem, 128)  # batch 2
                        first_u1[0] = False
                    for k in (2 * m, 2 * m + 1):
                        if 2 * m + 3 >= NPAIR:
                            # rbf waits absent near the tail: wait mm1 directly
                            act.wait_ge(pe_sem, PEC[f"mm1_{k}b"])
                        act.activation(
                            u1_sb[:, (k % 2) * CH : (k % 2 + 1) * CH],
                            mm1_ps[k % 2][:], AF.Exp, bias=b1p_sb[:],
                        ).then_inc(act_sem, 1)
                    if m >= 1:
                        act.wait_ge(pe_sem, PEC[f"mm2_{2 * m - 1}b"])
                        if m >= 2:  # u2_sb WAR vs f2(m-2) read (same engine)
                            act.wait_ge(act_sem, ACTC[f"f2_{m - 2}"])
                        for k in (2 * m - 2, 2 * m - 1):
                            act.activation(
                                u2_sb[:, (k % 2) * CH : (k % 2 + 1) * CH],
                                mm2_ps[k % 2][:], AF.Exp, bias=b2p_sb[:],
                            ).then_inc(act_sem, 1)
                    # f1' = ln(e^-k u1 + e^-k) = softplus(x1) - kappa, bf16
                    if first_f1[0]:
                        act.wait_ge(dma_sem, 128)  # batch 2
                        first_f1[0] = False
                    act.wait_ge(act_sem, ACTC[f"u1_{2 * m + 1}"])
                    act.activation(
                        f1_sb[:], u1_sb[:], AF.Ln, bias=emk_sb[:], scale=EMK
                    ).then_inc(act_sem, 1)
                    if m >= 1:
                        act.wait_ge(act_sem, ACTC[f"u2_{2 * m - 1}"])
                        if m >= 3:  # f2_sb[(m-1)%2] freed by DVE mul(m-3)
                            act.wait_ge(dve_sem, DVEC[f"mul{m - 3}"])
                        act.activation(
                            f2_sb[(m - 1) % 2][:], u2_sb[:], AF.Ln, bias=1.0
                        ).then_inc(act_sem, 1)
                # epilogue: u2 for the last two pairs, then f2(NM-1)
                act.wait_ge(pe_sem, PEC[f"mm2_{2 * NM - 1}b"])
                act.wait_ge(act_sem, ACTC[f"f2_{NM - 2}"])
                for k in (2 * NM - 2, 2 * NM - 1):
                    act.activation(
                        u2_sb[:, (k % 2) * CH : (k % 2 + 1) * CH],
                        mm2_ps[k % 2][:], AF.Exp, bias=b2p_sb[:],
                    ).then_inc(act_sem, 1)
                act.wait_ge(act_sem, ACTC[f"u2_{2 * NM - 1}"])
                act.wait_ge(dve_sem, DVEC[f"mul{NM - 3}"])
                act.activation(
                    f2_sb[(NM - 1) % 2][:], u2_sb[:], AF.Ln, bias=1.0
                ).then_inc(act_sem, 1)

            @block.vector
            def _(ve):
                ve.wait_ge(dma_sem, 16 * LOADS)   # hs (last load)
                for m in range(NM):
                    p = m % 2
                    b = m // (NM // MBC)
                    ve.wait_ge(act_sem, ACTC[f"f2_{m}"])
                    if m >= 2:  # prod_sb[p] freed by red(m-2)
                        ve.wait_ge(dve_sem, DVEC[f"red{m - 2}"])
                    ve.tensor_mul(
                        prod_sb[p][:],
                        f2_sb[p][:].rearrange("p (j i) -> p j i", i=ATOM),
                        hs_sb[:, b * ATOM : (b + 1) * ATOM][:, None, :].broadcast_to(
                            [128, 16, ATOM]
                        ),
                    ).then_inc(dve_sem, 1)
                    ve.wait_ge(dve_sem, DVEC[f"mul{m}"])
                    ve.reduce_sum(
                        res_sb[:, m * 16 : (m + 1) * 16],
                        prod_sb[p][:],
                        axis=mybir.AxisListType.X,
                    ).then_inc(dve_sem, 1)

    return nc


def _split_bf16(x, n):
    """Split fp32 array into n bf16 components summing to ~x."""
    import ml_dtypes

    bf = ml_dtypes.bfloat16
    x = x.astype(np.float32)
    parts = []
    for _ in range(n):
        p = x.astype(bf)
        parts.append(p)
        x = x - p.astype(np.float32)
    return parts


def host_prep(h, dist, W1, b1, W2, b2):
    """Build per-core input maps (numpy only, layout/index prep)."""
    import ml_dtypes

    bf = ml_dtypes.bfloat16
    f4 = np.float32
    r = np.arange(R, dtype=f4)
    coef = np.stack(
        [np.full(R, -GAMMA, f4)] * 3 + [(2.0 * r).astype(f4)] * 3
    ).astype(bf)
    bexp = (-GAMMA * (RES * r) ** 2).astype(f4)[:, None]
    w1b = W1[:R].astype(f4).astype(bf)
    b1p = np.concatenate([b1, b1]).astype(f4)[:, None]
    w2b = W2.astype(f4).astype(bf)
    w2d = np.ascontiguousarray(np.concatenate([w2b, w2b], 0))
    # kappa compensation: out2 = W2dev.T @ (f1 - kappa) + b2 + kappa*colsum(W2dev)
    b2c = (b2 + KAPPA * w2b.astype(f4).sum(0)).astype(f4)
    b2p = np.concatenate([b2c, b2c]).astype(f4)[:, None]

    in_maps = []
    for g in range(NCORES):
        dist_c = dist[g * MBC : (g + 1) * MBC].astype(f4)
        dperm = np.ascontiguousarray(dist_c.transpose(0, 2, 1)).reshape(-1)  # (b,j,i)
        d2 = (dperm * dperm).astype(f4)
        ddv = np.ascontiguousarray(np.stack(_split_bf16(d2, 3) + _split_bf16(dperm, 3)))
        h_c = h[g * MBC : (g + 1) * MBC].astype(f4)
        ht = np.ascontiguousarray(h_c.transpose(2, 0, 1)).reshape(HD, MBC * ATOM)
        hsv = np.ascontiguousarray(np.concatenate([ht, ht], 0))
        in_maps.append(
            {
                "dd": ddv, "coef": coef, "bexp": bexp,
                "w1": w1b, "b1p": b1p, "w2": w2d, "b2p": b2p, "hs": hsv,
                "emk": np.full((128, 1), np.exp(-KAPPA), f4),
            }
        )
    return in_maps


def decode_res(res_np):
    """res [128, 128] -> out_core [MBC, ATOM(j), HD(c)].

    res[cc, t*8+jl]: b = t//4, sig = t%4, j = 16*sig + 8*(cc>=64) + jl,
    c = cc % 64.
    """
    r5 = res_np.reshape(2, HD, MBC, NPAIR // MBC, 8)  # [half, c, b, sig, jl]
    return np.ascontiguousarray(r5.transpose(2, 3, 0, 4, 1)).reshape(MBC, ATOM, HD)


def kernel(h, dist, W1, b1, W2, b2):
    from concourse.bass_utils import run_bass_kernel_spmd

    if "nc" not in _CACHE:
        _CACHE["nc"] = build_bass()
    nc = _CACHE["nc"]
    in_maps = host_prep(h, dist, W1, b1, W2, b2)
    out = run_bass_kernel_spmd(nc, in_maps, list(range(NCORES)))
    cores = [decode_res(out.results[g]["res"]) for g in range(NCORES)]
    return np.concatenate(cores, axis=0).astype(np.float32)


# revision 24
# speedup vs baseline: 2.3724x; 1.0537x over previous
"""CFConv (SchNet continuous-filter convolution) Trainium2 kernel.

Math (per molecule b):
    rbf[b,i,j,r] = exp(-gamma * (dist[b,i,j] - r*res)^2),  r = 0..299
    f = softplus(rbf @ W1 + b1); f = softplus(f @ W2 + b2)
    out[b,j,c] = sum_i h[b,i,c] * f[b,i,j,c]

Device-level reformulations:
  * dist < 10.0 and gamma=10 => centers r >= 128 (c_r >= 12.8) give
    exp(<= -78) ~ 1e-34: identically negligible in fp32. K: 300 -> 128.
  * -g(d-c)^2 = (-g)*d^2 + (2gc)*d + (-g c^2): the first two terms form a
    matmul over host-provided rows; the per-r constant is the per-partition
    bias of the Exp activation. fp32 matmul on this PE runs in slow
    LOW_HIGH emulation (~2.1us per 512-col op), so all matmuls use bf16:
      - expansion: d and d^2 are each split into 3 bf16 components (K=6).
        The coefficients -10 and 2r (integers < 256) are EXACT in bf16,
        so every product is exact; residual ~2e-4 in the exponent.
      - mm1/mm2: bf16 weights and activations (PE runs at the cold
        1.2 GHz clock here, ~0.83 ns/column; fp32 would double the MMs
        again for ~3e-4 accuracy we don't need against the ~2e-2 gate).
        f1 is stored bf16 after a range shift: f1' = softplus(x1) - kappa
        via Ln(e^-k * u1 + e^-k), which halves the bf16 absolute error;
        kappa is compensated in b2.
  * softplus(x) = ln(exp(x) + 1) via Exp then Ln activations (both live in
    the same ACT table set; no native softplus table is deployed).
  * Elements are flattened in (b, j, i) order so the final contraction
    over i is a native inner-axis vector reduce per 512-element chunk.
  * Channel dim is 64; two 512-element chunks are stacked to fill all 128
    partitions for mm1/softplus/mm2/softplus/mul/reduce.

Raw Bass (no Tile): the deployed walrus accepts at most one sync-wait per
instruction, so all cross-engine deps are standalone single-condition
wait_ge instructions; buffers are double-buffered with parity t % 2, and
same-engine dependent ACT ops are separated by an independent op so the
self-wait is nearly free.

Sharding: data-parallel over mb across 8 cores (4 molecules/core), params
replicated. No collectives; host splits inputs and reassembles outputs.
"""

import numpy as np

MB, ATOM, HD = 32, 64, 64
R = 128                     # effective RBF count (of 300)
GAMMA, RES = 10.0, 0.1
KAPPA = 0.875               # f1 range shift (exact in bf16)
NCORES = 8
MBC = MB // NCORES          # molecules per core
E = MBC * ATOM * ATOM       # flattened (b, j, i) elements per core
CH = 512                    # e-chunk (one PSUM bank col-width)
NCHUNK = E // CH
NPAIR = NCHUNK // 2

_CACHE = {}


def build_bass():
    from contextlib import ExitStack

    import concourse.bass as bass
    from concourse import mybir

    f32 = mybir.dt.float32
    bf16 = mybir.dt.bfloat16
    AF = mybir.ActivationFunctionType

    NM = NPAIR // 2  # macro-iterations of 2 pairs (4 chunks, 2048 elems)

    nc = bass.Bass()
    dd = nc.declare_dram_parameter("dd", [6, E], bf16, isOutput=False)
    coef = nc.declare_dram_parameter("coef", [6, R], bf16, isOutput=False)
    bexp = nc.declare_dram_parameter("bexp", [R, 1], f32, isOutput=False)
    w1 = nc.declare_dram_parameter("w1", [R, HD], bf16, isOutput=False)
    b1p = nc.declare_dram_parameter("b1p", [128, 1], f32, isOutput=False)
    w2 = nc.declare_dram_parameter("w2", [2 * HD, HD], bf16, isOutput=False)
    b2p = nc.declare_dram_parameter("b2p", [128, 1], f32, isOutput=False)
    hs = nc.declare_dram_parameter("hs", [128, MBC * ATOM], f32, isOutput=False)
    emk = nc.declare_dram_parameter("emk", [128, 1], f32, isOutput=False)
    res = nc.declare_dram_parameter("res", [128, NPAIR * 8], f32, isOutput=True)

    EMK = float(np.exp(-KAPPA))

    with ExitStack() as ctx:
        en = ctx.enter_context

        dd_sb = en(nc.sbuf_tensor("dd_sb", [6, E], bf16))
        coef_sb = en(nc.sbuf_tensor("coef_sb", [6, R], bf16))
        bexp_sb = en(nc.sbuf_tensor("bexp_sb", [R, 1], f32))
        w1_sb = en(nc.sbuf_tensor("w1_sb", [R, HD], bf16))
        b1p_sb = en(nc.sbuf_tensor("b1p_sb", [128, 1], f32))
        w2_sb = en(nc.sbuf_tensor("w2_sb", [2 * HD, HD], bf16))
        b2p_sb = en(nc.sbuf_tensor("b2p_sb", [128, 1], f32))
        hs_sb = en(nc.sbuf_tensor("hs_sb", [128, MBC * ATOM], f32))
        emk_sb = en(nc.sbuf_tensor("emk_sb", [128, 1], f32))
        res_sb = en(nc.sbuf_tensor("res_sb", [128, NPAIR * 8], f32))

        # per-pair rbf tiles (parity k%2); per-macro layer tiles
        rbf_sb = [en(nc.sbuf_tensor(f"rbf{i}", [128, 2 * CH], bf16)) for i in (0, 1)]
        u1_sb = en(nc.sbuf_tensor("u1_sb", [128, 2 * CH], f32))
        f1_sb = en(nc.sbuf_tensor("f1_sb", [128, 2 * CH], bf16))
        u2_sb = en(nc.sbuf_tensor("u2_sb", [128, 2 * CH], f32))
        f2_sb = [en(nc.sbuf_tensor(f"f2_{i}", [128, 2 * CH], f32)) for i in (0, 1)]
        prod_sb = [en(nc.sbuf_tensor(f"prod{i}", [128, 16, ATOM], f32)) for i in (0, 1)]

        exp_ps = [en(nc.psum_tensor(f"expps{i}", [128, 2 * CH], f32)) for i in (0, 1)]
        mm1_ps = [en(nc.psum_tensor(f"mm1ps{i}", [128, CH], f32)) for i in (0, 1)]
        mm2_ps = [en(nc.psum_tensor(f"mm2ps{i}", [128, CH], f32)) for i in (0, 1)]

        dma_sem = en(nc.semaphore("dma_sem"))
        dma2_sem = en(nc.semaphore("dma2_sem"))
        pe_sem = en(nc.semaphore("pe_sem"))
        act_sem = en(nc.semaphore("act_sem"))
        dve_sem = en(nc.semaphore("dve_sem"))

        LOADS = 9  # input DMA transfers

        # ---- software-pipelined schedule (macro m = pairs 2m, 2m+1) ----
        # PE:  exp(0..3), mm1(0), mm1(1)
        #      | per m: exp(2m+4), exp(2m+5), mm1(2m+2), mm1(2m+3),
        #               mm2(2m), mm2(2m+1)
        # ACT: rbf(0), rbf(1)
        #      | per m: rbf(2m+2), rbf(2m+3), u1(2m), u1(2m+1),
        #               u2(2m-2), u2(2m-1), f1(m), f2(m-1)
        #      | u2(2NM-2), u2(2NM-1), f2(NM-1)
        # mm1 runs one macro ahead of f1/mm2; u2/f2 lag one macro, so the
        # mm1->u1->f1->mm2 chain of macro m overlaps macro m+1's mm1.
        def seq_counts(names):
            return {n: i + 1 for i, n in enumerate(names)}

        pe_ops = ["exp0a", "exp0b", "exp1a", "exp1b", "exp2a", "exp2b",
                  "exp3a", "exp3b", "mm1_0a", "mm1_0b", "mm1_1a", "mm1_1b"]
        for m in range(NM):
            for k in (2 * m + 4, 2 * m + 5):
                if k < NPAIR:
                    pe_ops += [f"exp{k}a", f"exp{k}b"]
            for k in (2 * m + 2, 2 * m + 3):
                if k < NPAIR:
                    pe_ops += [f"mm1_{k}a", f"mm1_{k}b"]
            pe_ops += [f"mm2_{2 * m}a", f"mm2_{2 * m}b",
                       f"mm2_{2 * m + 1}a", f"mm2_{2 * m + 1}b"]
        PEC = seq_counts(pe_ops)

        act_ops = ["rbf0", "rbf1"]
        for m in range(NM):
            for k in (2 * m + 2, 2 * m + 3):
                if k < NPAIR:
                    act_ops.append(f"rbf{k}")
            act_ops += [f"u1_{2 * m}", f"u1_{2 * m + 1}"]
            if m >= 1:
                act_ops += [f"u2_{2 * m - 2}", f"u2_{2 * m - 1}"]
            act_ops.append(f"f1_{m}")
            if m >= 1:
                act_ops.append(f"f2_{m - 1}")
        act_ops += [f"u2_{2 * NM - 2}", f"u2_{2 * NM - 1}", f"f2_{NM - 1}"]
        ACTC = seq_counts(act_ops)

        dve_ops = []
        for m in range(NM):
            dve_ops += [f"mul{m}", f"red{m}"]
        DVEC = seq_counts(dve_ops)

        with nc.Block() as block:

            @block.sync
            def _(sy):
                # big tensors on HWDGE, piece-serialized for stable counts
                PIECE = E // 4
                for i in range(4):
                    sy.dma_start(
                        dd_sb[:, i * PIECE : (i + 1) * PIECE],
                        dd[:, i * PIECE : (i + 1) * PIECE],
                    ).then_inc(dma2_sem, 16)
                    sy.wait_ge(dma2_sem, 16 * (i + 1))
                sy.dma_start(hs_sb[:], hs[:]).then_inc(dma2_sem, 16)

            @block.gpsimd
            def _(g):
                # small loads; batch boundaries are stable wait points
                for dst, src_ in [(coef_sb, coef), (bexp_sb, bexp)]:
                    g.dma_start(dst[:], src_[:]).then_inc(dma_sem, 16)
                g.wait_ge(dma_sem, 32)
                for dst, src_ in [(w1_sb, w1), (b1p_sb, b1p), (w2_sb, w2),
                                  (b2p_sb, b2p), (emk_sb, emk)]:
                    g.dma_start(dst[:], src_[:]).then_inc(dma_sem, 16)
                # output store after the last reduce
                g.wait_ge(dve_sem, DVEC[f"red{NM - 1}"])
                g.dma_start(res[:], res_sb[:]).then_inc(dma_sem, 16)
                g.wait_ge(dma_sem, 16 * 8)

            def emit_exp_mm(pe, k):
                p = k % 2
                for half in range(2):
                    q = 2 * k + half
                    pe.matmul(
                        exp_ps[p][:, half * CH : (half + 1) * CH],
                        coef_sb[:],
                        dd_sb[:, q * CH : (q + 1) * CH],
                        start=True, stop=True,
                    ).then_inc(pe_sem, 1)

            def emit_mm1(pe, k):
                p = k % 2
                pe.matmul(
                    mm1_ps[p][0:64, :], w1_sb[:], rbf_sb[p][:, 0:CH],
                    start=True, stop=True,
                ).then_inc(pe_sem, 1)
                pe.matmul(
                    mm1_ps[p][64:128, :], w1_sb[:], rbf_sb[p][:, CH : 2 * CH],
                    start=True, stop=True,
                ).then_inc(pe_sem, 1)

            def emit_mm2(pe, k):
                p = k % 2
                pe.matmul(
                    mm2_ps[p][0:64, :], w2_sb[0:64, :],
                    f1_sb[0:64, p * CH : (p + 1) * CH],
                    start=True, stop=True,
                ).then_inc(pe_sem, 1)
                pe.matmul(
                    mm2_ps[p][64:128, :], w2_sb[64:128, :],
                    f1_sb[64:128, p * CH : (p + 1) * CH],
                    start=True, stop=True,
                ).then_inc(pe_sem, 1)

            @block.tensor
            def _(pe):
                pe.wait_ge(dma_sem, 32)    # coef
                pe.wait_ge(dma2_sem, 16)   # dd piece 0
                emit_exp_mm(pe, 0)
                emit_exp_mm(pe, 1)
                pe.wait_ge(act_sem, ACTC["rbf0"])
                emit_exp_mm(pe, 2)
                pe.wait_ge(act_sem, ACTC["rbf1"])
                emit_exp_mm(pe, 3)
                pe.wait_ge(dma_sem, 16 * 7)  # weights/biases
                emit_mm1(pe, 0)  # rbf0/rbf1 waits subsumed above
                emit_mm1(pe, 1)
                for m in range(NM):
                    for k in (2 * m + 4, 2 * m + 5):
                        if k < NPAIR:
                            pe.wait_ge(act_sem, ACTC[f"rbf{k - 2}"])
                            if k % 4 == 0:
                                pe.wait_ge(dma2_sem, 16 * (k // 4 + 1))
                            emit_exp_mm(pe, k)
                    for k in (2 * m + 2, 2 * m + 3):
                        if k < NPAIR:
                            pe.wait_ge(act_sem, ACTC[f"u1_{k - 2}"])
                            emit_mm1(pe, k)
                    pe.wait_ge(act_sem, ACTC[f"f1_{m}"])
                    emit_mm2(pe, 2 * m)
                    emit_mm2(pe, 2 * m + 1)

            @block.scalar
            def _(act):
                act.wait_ge(dma_sem, 32)   # bexp
                first_u1 = [True]
                first_f1 = [True]

                def rbf_act(k, wait_mm1=True):
                    p = k % 2
                    if wait_mm1:
                        act.wait_ge(pe_sem, PEC[f"mm1_{k - 2}b"])
                    else:
                        act.wait_ge(pe_sem, PEC[f"exp{k}b"])
                    act.activation(
                        rbf_sb[p][:], exp_ps[p][:], AF.Exp, bias=bexp_sb[:]
                    ).then_inc(act_sem, 1)

                rbf_act(0, wait_mm1=False)
                rbf_act(1, wait_mm1=False)
                for m in range(NM):
                    for k in (2 * m + 2, 2 * m + 3):
                        if k < NPAIR:
                            rbf_act(k)
                    if m >= 1:  # u1_sb WAR vs f1(m-1) read (same engine)
                        act.wait_ge(act_sem, ACTC[f"f1_{m - 1}"])
                    if first_u1[0]:
                        act.wait_ge(dma_sem, 16 * 7)  # biases
                        first_u1[0] = False
                    for k in (2 * m, 2 * m + 1):
                        if 2 * m + 3 >= NPAIR:
                            # rbf waits absent near the tail: wait mm1 directly
                            act.wait_ge(pe_sem, PEC[f"mm1_{k}b"])
                        act.activation(
                            u1_sb[:, (k % 2) * CH : (k % 2 + 1) * CH],
                            mm1_ps[k % 2][:], AF.Exp, bias=b1p_sb[:],
                        ).then_inc(act_sem, 1)
                    if m >= 1:
                        act.wait_ge(pe_sem, PEC[f"mm2_{2 * m - 1}b"])
                        if m >= 2:  # u2_sb WAR vs f2(m-2) read (same engine)
                            act.wait_ge(act_sem, ACTC[f"f2_{m - 2}"])
                        for k in (2 * m - 2, 2 * m - 1):
                            act.activation(
                                u2_sb[:, (k % 2) * CH : (k % 2 + 1) * CH],
                                mm2_ps[k % 2][:], AF.Exp, bias=b2p_sb[:],
                            ).then_inc(act_sem, 1)
                    # f1' = ln(e^-k u1 + e^-k) = softplus(x1) - kappa, bf16
                    if first_f1[0]:
                        act.wait_ge(dma_sem, 16 * 7)  # emk
                        first_f1[0] = False
                    act.wait_ge(act_sem, ACTC[f"u1_{2 * m + 1}"])
                    act.activation(
                        f1_sb[:], u1_sb[:], AF.Ln, bias=emk_sb[:], scale=EMK
                    ).then_inc(act_sem, 1)
                    if m >= 1:
                        act.wait_ge(act_sem, ACTC[f"u2_{2 * m - 1}"])
                        if m >= 3:  # f2_sb[(m-1)%2] freed by DVE mul(m-3)
                            act.wait_ge(dve_sem, DVEC[f"mul{m - 3}"])
                        act.activation(
                            f2_sb[(m - 1) % 2][:], u2_sb[:], AF.Ln, bias=1.0
                        ).then_inc(act_sem, 1)
                # epilogue: u2 for the last two pairs, then f2(NM-1)
                act.wait_ge(pe_sem, PEC[f"mm2_{2 * NM - 1}b"])
                act.wait_ge(act_sem, ACTC[f"f2_{NM - 2}"])
                for k in (2 * NM - 2, 2 * NM - 1):
                    act.activation(
                        u2_sb[:, (k % 2) * CH : (k % 2 + 1) * CH],
                        mm2_ps[k % 2][:], AF.Exp, bias=b2p_sb[:],
                    ).then_inc(act_sem, 1)
                act.wait_ge(act_sem, ACTC[f"u2_{2 * NM - 1}"])
                act.wait_ge(dve_sem, DVEC[f"mul{NM - 3}"])
                act.activation(
                    f2_sb[(NM - 1) % 2][:], u2_sb[:], AF.Ln, bias=1.0
                ).then_inc(act_sem, 1)

            @block.vector
            def _(ve):
                ve.wait_ge(dma2_sem, 16 * 5)   # hs
                for m in range(NM):
                    p = m % 2
                    b = m // (NM // MBC)
                    ve.wait_ge(act_sem, ACTC[f"f2_{m}"])
                    if m >= 2:  # prod_sb[p] freed by red(m-2)
                        ve.wait_ge(dve_sem, DVEC[f"red{m - 2}"])
                    ve.tensor_mul(
                        prod_sb[p][:],
                        f2_sb[p][:].rearrange("p (j i) -> p j i", i=ATOM),
                        hs_sb[:, b * ATOM : (b + 1) * ATOM][:, None, :].broadcast_to(
                            [128, 16, ATOM]
                        ),
                    ).then_inc(dve_sem, 1)
                    ve.wait_ge(dve_sem, DVEC[f"mul{m}"])
                    ve.reduce_sum(
                        res_sb[:, m * 16 : (m + 1) * 16],
                        prod_sb[p][:],
                        axis=mybir.AxisListType.X,
                    ).then_inc(dve_sem, 1)

    return nc


def _split_bf16(x, n):
    """Split fp32 array into n bf16 components summing to ~x."""
    import ml_dtypes

    bf = ml_dtypes.bfloat16
    x = x.astype(np.float32)
    parts = []
    for _ in range(n):
        p = x.astype(bf)
        parts.append(p)
        x = x - p.astype(np.float32)
    return parts


def host_prep(h, dist, W1, b1, W2, b2):
    """Build per-core input maps (numpy only, layout/index prep)."""
    import ml_dtypes

    bf = ml_dtypes.bfloat16
    f4 = np.float32
    r = np.arange(R, dtype=f4)
    coef = np.stack(
        [np.full(R, -GAMMA, f4)] * 3 + [(2.0 * r).astype(f4)] * 3
    ).astype(bf)
    bexp = (-GAMMA * (RES * r) ** 2).astype(f4)[:, None]
    w1b = W1[:R].astype(f4).astype(bf)
    b1p = np.concatenate([b1, b1]).astype(f4)[:, None]
    w2b = W2.astype(f4).astype(bf)
    w2d = np.ascontiguousarray(np.concatenate([w2b, w2b], 0))
    # kappa compensation: out2 = W2dev.T @ (f1 - kappa) + b2 + kappa*colsum(W2dev)
    b2c = (b2 + KAPPA * w2b.astype(f4).sum(0)).astype(f4)
    b2p = np.concatenate([b2c, b2c]).astype(f4)[:, None]

    in_maps = []
    for g in range(NCORES):
        dist_c = dist[g * MBC : (g + 1) * MBC].astype(f4)
        dperm = np.ascontiguousarray(dist_c.transpose(0, 2, 1)).reshape(-1)  # (b,j,i)
        d2 = (dperm * dperm).astype(f4)
        ddv = np.ascontiguousarray(np.stack(_split_bf16(d2, 3) + _split_bf16(dperm, 3)))
        h_c = h[g * MBC : (g + 1) * MBC].astype(f4)
        ht = np.ascontiguousarray(h_c.transpose(2, 0, 1)).reshape(HD, MBC * ATOM)
        hsv = np.ascontiguousarray(np.concatenate([ht, ht], 0))
        in_maps.append(
            {
                "dd": ddv, "coef": coef, "bexp": bexp,
                "w1": w1b, "b1p": b1p, "w2": w2d, "b2p": b2p, "hs": hsv,
                "emk": np.full((128, 1), np.exp(-KAPPA), f4),
            }
        )
    return in_maps


def decode_res(res_np):
    """res [128, 128] -> out_core [MBC, ATOM(j), HD(c)].

    res[cc, t*8+jl]: b = t//4, sig = t%4, j = 16*sig + 8*(cc>=64) + jl,
    c = cc % 64.
    """
    r5 = res_np.reshape(2, HD, MBC, NPAIR // MBC, 8)  # [half, c, b, sig, jl]
    return np.ascontiguousarray(r5.transpose(2, 3, 0, 4, 1)).reshape(MBC, ATOM, HD)


def kernel(h, dist, W1, b1, W2, b2):
    from concourse.bass_utils import run_bass_kernel_spmd

    if "nc" not in _CACHE:
        _CACHE["nc"] = build_bass()
    nc = _CACHE["nc"]
    in_maps = host_prep(h, dist, W1, b1, W2, b2)
    out = run_bass_kernel_spmd(nc, in_maps, list(range(NCORES)))
    cores = [decode_res(out.results[g]["res"]) for g in range(NCORES)]
    return np.concatenate(cores, axis=0).astype(np.float32)
